# revision 1
# baseline (speedup 1.0000x reference)
"""Trainium2 Bass kernel for ModalityAttention (B=4, S=1024, D=2048, H=16, HD=128, RD=64).

Sharding: 8 cores = 4 batches x 2 head-groups (8 heads each).
Each core computes, for its (batch b, head-group g):
  layernorm(x[b]) -> modulation (scale/bias precomputed on host from mod@mod_w)
  -> qkv projection for its 8 heads -> rmsnorm + rope -> attention
  -> partial out-projection (transposed layout) with gate folded in.
Host gathers: out[b] = (partial_g0 + partial_g1).T + x[b]
(residual added on host; vb = out_b*gate folded into the g0 partial on device).
"""
import os, sys

for _p in ("/opt/trn_rl_repo", "/root/.axon_site/_ro/trn_rl_repo", "/root/.axon_site"):
    if os.path.isdir(_p) and _p not in sys.path:
        sys.path.insert(0, _p)

import numpy as np
import concourse.bass as bass
import concourse.bacc as bacc
import concourse.mybir as mybir
import concourse.tile as tile
from concourse import bass_isa
from concourse.masks import make_identity
from concourse.bass_utils import run_bass_kernel_spmd

F32 = mybir.dt.float32
AF = mybir.ActivationFunctionType
S, D, HG, HD, RD = 1024, 2048, 8, 128, 64
NT = S // 128        # 8 s-tiles
KT = D // 128        # 16 d-tiles
GCOLS = HG * HD      # 1024 columns per group per projection
EPS = 1e-6
N_CORES = 8


def _bcast_from_dram(ap, parts, reps=None):
    """DRAM AP -> partition-broadcast (and optional middle-dim repeat) source AP."""
    newap = [[0, parts]]
    if reps is not None:
        newap.append([0, reps])
    newap += list(ap.ap)
    return bass.AP(tensor=ap.tensor, offset=ap.offset, ap=newap)


def build_nc(has_qkv_bias: bool, has_norm_w: bool):
    nc = bacc.Bacc("TRN2", target_bir_lowering=False, debug=False,
                   enable_asserts=True, num_devices=N_CORES)

    x = nc.dram_tensor("x", [S, D], F32, kind="ExternalInput").ap()
    cos = nc.dram_tensor("cos", [S, RD // 2], F32, kind="ExternalInput").ap()
    sin = nc.dram_tensor("sin", [S, RD // 2], F32, kind="ExternalInput").ap()
    wq = nc.dram_tensor("wq", [D, GCOLS], F32, kind="ExternalInput").ap()
    wk = nc.dram_tensor("wk", [D, GCOLS], F32, kind="ExternalInput").ap()
    wv = nc.dram_tensor("wv", [D, GCOLS], F32, kind="ExternalInput").ap()
    wo = nc.dram_tensor("wo", [GCOLS, D], F32, kind="ExternalInput").ap()
    # modulation vectors, pre-reshaped on host to [128, KT] (column k = d-tile k)
    scale1p = nc.dram_tensor("scale1p", [128, KT], F32, kind="ExternalInput").ap()
    biasm = nc.dram_tensor("biasm", [128, KT], F32, kind="ExternalInput").ap()
    gate = nc.dram_tensor("gate", [128, KT], F32, kind="ExternalInput").ap()
    vb = nc.dram_tensor("vb", [128, KT], F32, kind="ExternalInput").ap()
    if has_qkv_bias:
        bq = nc.dram_tensor("bq", [GCOLS], F32, kind="ExternalInput").ap()
        bk = nc.dram_tensor("bk", [GCOLS], F32, kind="ExternalInput").ap()
        bv = nc.dram_tensor("bv", [GCOLS], F32, kind="ExternalInput").ap()
    if has_norm_w:
        wqn = nc.dram_tensor("wqn", [HD], F32, kind="ExternalInput").ap()
        wkn = nc.dram_tensor("wkn", [HD], F32, kind="ExternalInput").ap()
    out_t = nc.dram_tensor("out_t", [D, S], F32, kind="ExternalOutput").ap()

    with tile.TileContext(nc) as tc:
        # ======== LEFT stack bottom: small persistent constants ====================
        misc_cm = tc.tile_pool(name="misc", bufs=1, side="left")
        misc = misc_cm.__enter__()
        ident = misc.tile([128, 128], F32)
        make_identity(nc, ident)
        ones_col = misc.tile([128, 1], F32)
        nc.vector.memset(ones_col, 1.0)
        eps_t = misc.tile([128, 1], F32)
        nc.vector.memset(eps_t, EPS)
        eps128_t = misc.tile([128, 1], F32)
        nc.vector.memset(eps128_t, HD * EPS)
        gate_sb = misc.tile([128, KT], F32)
        vb_sb = misc.tile([128, KT], F32)
        rrk_all = misc.tile([128, NT, HG], F32)   # scaled k-rms reciprocals
        if has_norm_w:
            wqn_b = misc.tile([128, HG, HD], F32)
            wkn_b = misc.tile([128, HG, HD], F32)
        cs_tiles = []
        for m in range(NT):
            ct = misc.tile([128, RD // 2], F32, tag=f"cos_{m}", name=f"cos_{m}")
            st = misc.tile([128, RD // 2], F32, tag=f"sin_{m}", name=f"sin_{m}")
            cs_tiles.append((ct, st))
        # (misc DMAs are emitted after phase A so the x-tile loads go first
        #  in the HWDGE queue; these tiles are only consumed in later phases)

        # ======== RIGHT stack: big natural-layout tensors (B..E lifetimes) =========
        v_cm = tc.tile_pool(name="vpool", bufs=1, side="right")
        v_p = v_cm.__enter__()
        vnat = v_p.tile([128, NT, GCOLS], F32)
        natqk_cm = tc.tile_pool(name="natqk", bufs=1, side="right")
        natqk = natqk_cm.__enter__()
        qnat = natqk.tile([128, NT, GCOLS], F32)
        knat = natqk.tile([128, NT, GCOLS], F32)

        # ======== phase A: layernorm + modulation + transpose -> xnT ===============
        xnT_cm = tc.tile_pool(name="xnT", bufs=1, side="left")
        xnT_p = xnT_cm.__enter__()
        xnT = xnT_p.tile([128, KT, S], F32)  # [d_in_tile, d_tile, s]

        avec_cm = tc.tile_pool(name="phA_vec", bufs=1, side="left")
        avec = avec_cm.__enter__()
        s1pc = avec.tile([128, KT], F32)
        bmc = avec.tile([128, KT], F32)
        if has_qkv_bias:
            bq_b = avec.tile([128, GCOLS], F32)
            nc.sync.dma_start(out=bq_b, in_=_bcast_from_dram(bq, 128))
            bk_b = avec.tile([128, GCOLS], F32)
            nc.sync.dma_start(out=bk_b, in_=_bcast_from_dram(bk, 128))
            bv_b = avec.tile([128, GCOLS], F32)
            nc.sync.dma_start(out=bv_b, in_=_bcast_from_dram(bv, 128))

        a_cm = tc.tile_pool(name="phA", bufs=3, side="left")
        a_p = a_cm.__enter__()
        a_small_cm = tc.tile_pool(name="phA_small", bufs=4, side="left")
        a_small = a_small_cm.__enter__()
        pst_cm = tc.tile_pool(name="ps_tr", bufs=4, space="PSUM")
        pst = pst_cm.__enter__()

        for i in range(NT):
            xt = a_p.tile([128, D], F32, tag="xt")
            nc.sync.dma_start(out=xt, in_=x[i * 128:(i + 1) * 128, :])
            if i == 0:
                nc.sync.dma_start(out=s1pc, in_=scale1p)
                nc.sync.dma_start(out=bmc, in_=biasm)
            stats = a_small.tile([128, 4, 6], F32, tag="stats")
            xv = xt.rearrange("p (c f) -> p c f", c=4)
            for c in range(4):
                nc.vector.bn_stats(out=stats[:, c, :], in_=xv[:, c, :])
            mv = a_small.tile([128, 2], F32, tag="mv")
            nc.vector.bn_aggr(out=mv, in_=stats)
            rstd = a_small.tile([128, 1], F32, tag="rstd")
            nc.scalar.activation(out=rstd, in_=mv[:, 1:2], func=AF.Sqrt,
                                 bias=eps_t, scale=1.0)
            nc.vector.reciprocal(out=rstd, in_=rstd)
            nmr = a_small.tile([128, 1], F32, tag="nmr")
            nc.vector.tensor_mul(out=nmr, in0=mv[:, 0:1], in1=rstd)
            nc.scalar.mul(out=nmr, in_=nmr, mul=-1.0)
            nc.scalar.activation(out=xt, in_=xt, func=AF.Identity,
                                 bias=nmr, scale=rstd)
            for k in range(KT):
                pt = pst.tile([128, 128], F32, tag="pt")
                nc.tensor.transpose(pt, xt[:, k * 128:(k + 1) * 128], ident)
                # modulation fused into the evac: xnT = pt * (1+scale[d]) + bias[d]
                nc.scalar.activation(out=xnT[:, k, i * 128:(i + 1) * 128], in_=pt,
                                     func=AF.Identity,
                                     bias=bmc[:, k:k + 1], scale=s1pc[:, k:k + 1])

        # deferred misc loads (consumed in phases C/E/F)
        nc.sync.dma_start(out=gate_sb, in_=gate)
        nc.sync.dma_start(out=vb_sb, in_=vb)
        if has_norm_w:
            nc.sync.dma_start(out=wqn_b, in_=_bcast_from_dram(wqn, 128, reps=HG))
            nc.sync.dma_start(out=wkn_b, in_=_bcast_from_dram(wkn, 128, reps=HG))
        for m in range(NT):
            ct, st = cs_tiles[m]
            nc.sync.dma_start(out=ct, in_=cos[m * 128:(m + 1) * 128, :])
            nc.sync.dma_start(out=st, in_=sin[m * 128:(m + 1) * 128, :])

        pst_cm.__exit__(None, None, None)
        a_small_cm.__exit__(None, None, None)
        a_cm.__exit__(None, None, None)

        # phase C pools opened BEFORE phase B emission so the rms/rope work can
        # overlap the tail of the qkv matmuls (no pool-boundary serialization).
        # With qkv biases present SBUF is too tight for the overlap; in that
        # case C pools open after B instead.
        overlap_c = not has_qkv_bias
        if overlap_c:
            c_cm = tc.tile_pool(name="phC", bufs=2, side="left")
            c_p = c_cm.__enter__()
            c_small_cm = tc.tile_pool(name="phC_small", bufs=2, side="left")
            c_small = c_small_cm.__enter__()

        # ======== phase B: qkv projections (natural layout) ========================
        w_cm = tc.tile_pool(name="wstream", bufs=3, side="right")
        w_p = w_cm.__enter__()
        psb_cm = tc.tile_pool(name="ps_qkv", bufs=1, space="PSUM")
        psb = psb_cm.__enter__()

        for (wdram, nat) in ((wq, qnat), (wk, knat), (wv, vnat)):
            for n in range(2):
                ps = [psb.tile([128, 512], F32, tag=f"ps{m}", name=f"ps{m}")
                      for m in range(NT)]
                for k in range(KT):
                    wt = w_p.tile([128, 512], F32, tag="wt")
                    nc.sync.dma_start(
                        out=wt, in_=wdram[k * 128:(k + 1) * 128, n * 512:(n + 1) * 512])
                    for m in range(NT):
                        nc.tensor.matmul(ps[m], xnT[:, k, m * 128:(m + 1) * 128], wt,
                                         start=(k == 0), stop=(k == KT - 1))
                for m in range(NT):
                    nc.scalar.copy(out=nat[:, m, n * 512:(n + 1) * 512], in_=ps[m])
        if has_qkv_bias:
            for m in range(NT):
                nc.gpsimd.tensor_add(out=qnat[:, m, :], in0=qnat[:, m, :], in1=bq_b)
                nc.gpsimd.tensor_add(out=knat[:, m, :], in0=knat[:, m, :], in1=bk_b)
                nc.gpsimd.tensor_add(out=vnat[:, m, :], in0=vnat[:, m, :], in1=bv_b)

        psb_cm.__exit__(None, None, None)
        w_cm.__exit__(None, None, None)

        # ======== phase C: rmsnorm + rope on q, k (natural, in place) ==============
        if not overlap_c:
            c_cm = tc.tile_pool(name="phC", bufs=2, side="left")
            c_p = c_cm.__enter__()
            c_small_cm = tc.tile_pool(name="phC_small", bufs=2, side="left")
            c_small = c_small_cm.__enter__()

        for m in range(NT):
            qm = qnat[:, m, :]
            km = knat[:, m, :]
            (ct, st) = cs_tiles[m]
            cb = ct.unsqueeze(1).broadcast_to([128, HG, RD // 2])
            sb_ = st.unsqueeze(1).broadcast_to([128, HG, RD // 2])

            # rms stats (on raw q/k, before norm-w and rope)
            sq = c_p.tile([128, GCOLS], F32, tag="sqk")
            nc.vector.tensor_mul(out=sq, in0=qm, in1=qm)
            ssq = c_small.tile([128, HG], F32, tag="ssq")
            nc.vector.reduce_sum(out=ssq, in_=sq.rearrange("p (h d) -> p h d", h=HG),
                                 axis=mybir.AxisListType.X)
            rrq = c_small.tile([128, HG], F32, tag="rrq")
            nc.scalar.activation(out=rrq, in_=ssq, func=AF.Sqrt,
                                 bias=eps_t, scale=1.0 / HD)
            nc.vector.reciprocal(out=rrq, in_=rrq)

            sk_ = c_p.tile([128, GCOLS], F32, tag="sqk")
            nc.vector.tensor_mul(out=sk_, in0=km, in1=km)
            ssk = c_small.tile([128, HG], F32, tag="ssk")
            nc.vector.reduce_sum(out=ssk, in_=sk_.rearrange("p (h d) -> p h d", h=HG),
                                 axis=mybir.AxisListType.X)
            nc.scalar.activation(out=rrk_all[:, m, :], in_=ssk, func=AF.Sqrt,
                                 bias=eps128_t, scale=1.0)
            nc.vector.reciprocal(out=rrk_all[:, m, :], in_=rrk_all[:, m, :])

            if has_norm_w:
                nc.vector.tensor_mul(out=qm.rearrange("p (h d) -> p h d", h=HG),
                                     in0=qm.rearrange("p (h d) -> p h d", h=HG),
                                     in1=wqn_b)
                nc.vector.tensor_mul(out=km.rearrange("p (h d) -> p h d", h=HG),
                                     in0=km.rearrange("p (h d) -> p h d", h=HG),
                                     in1=wkn_b)

            for mm in (qm, km):
                mv_ = mm.rearrange("p (h i two) -> p h i two", h=HG, two=2)
                x0 = mv_[:, :, 0:RD // 2, 0]
                x1 = mv_[:, :, 0:RD // 2, 1]
                t0 = c_small.tile([128, HG, RD // 2], F32, tag="t0")
                t1 = c_small.tile([128, HG, RD // 2], F32, tag="t1")
                t2 = c_small.tile([128, HG, RD // 2], F32, tag="t2")
                t3 = c_small.tile([128, HG, RD // 2], F32, tag="t3")
                nc.vector.tensor_mul(out=t0, in0=x0, in1=cb)
                nc.vector.tensor_mul(out=t1, in0=x1, in1=sb_)
                nc.vector.tensor_mul(out=t2, in0=x0, in1=sb_)
                nc.vector.tensor_mul(out=t3, in0=x1, in1=cb)
                nc.gpsimd.tensor_sub(out=x0, in0=t0, in1=t1)
                nc.gpsimd.tensor_add(out=x1, in0=t2, in1=t3)

            # apply q rms reciprocal (k's is folded into the exp scale later)
            rrq_b = rrq.unsqueeze(2).broadcast_to([128, HG, HD])
            nc.vector.tensor_mul(out=qm.rearrange("p (h d) -> p h d", h=HG),
                                 in0=qm.rearrange("p (h d) -> p h d", h=HG),
                                 in1=rrq_b)

        c_small_cm.__exit__(None, None, None)
        c_cm.__exit__(None, None, None)
        avec_cm.__exit__(None, None, None)
        xnT_cm.__exit__(None, None, None)

        # ======== phases D/E/F share the left stack: oT under qkT ==================
        oT_cm = tc.tile_pool(name="oT", bufs=1, side="left")
        oT_p = oT_cm.__enter__()
        oT = oT_p.tile([128, HG, S], F32)

        # ---- phase D: transpose q, k -> [hd, s] per head
        qkT_cm = tc.tile_pool(name="qkT", bufs=1, side="left")
        qkT_p = qkT_cm.__enter__()
        qT = qkT_p.tile([128, HG, S], F32)
        kT = qkT_p.tile([128, HG, S], F32)
        pst2_cm = tc.tile_pool(name="ps_tr2", bufs=4, space="PSUM")
        pst2 = pst2_cm.__enter__()
        for (nat, dst) in ((qnat, qT), (knat, kT)):
            for h in range(HG):
                for m in range(NT):
                    pt2 = pst2.tile([128, 128], F32, tag="pt2")
                    nc.tensor.transpose(pt2, nat[:, m, h * 128:(h + 1) * 128], ident)
                    nc.scalar.copy(out=dst[:, h, m * 128:(m + 1) * 128], in_=pt2)
        pst2_cm.__exit__(None, None, None)
        natqk_cm.__exit__(None, None, None)

        # ---- phase E: attention per head
        at_cm = tc.tile_pool(name="attn", bufs=3, side="left")
        at_p = at_cm.__enter__()
        rs_cm = tc.tile_pool(name="rsb", bufs=2, side="left")
        rs_p = rs_cm.__enter__()
        pssc_cm = tc.tile_pool(name="ps_sc", bufs=3, space="PSUM")
        pssc = pssc_cm.__enter__()
        pso_cm = tc.tile_pool(name="ps_o", bufs=1, space="PSUM")
        pso = pso_cm.__enter__()

        for h in range(HG):
            o_ps = pso.tile([128, S], F32, tag="o_ps")
            acc = rs_p.tile([128, S], F32, tag="acc")
            for m in range(NT):
                sc = pssc.tile([128, S], F32, tag="sc")
                lhs_k = kT[:, h, m * 128:(m + 1) * 128]
                nc.tensor.matmul(sc[:, 0:512], lhs_k, qT[:, h, 0:512],
                                 start=True, stop=True)
                nc.tensor.matmul(sc[:, 512:1024], lhs_k, qT[:, h, 512:1024],
                                 start=True, stop=True)
                at = at_p.tile([128, S], F32, tag="at", name="at")
                nc.scalar.activation(out=at, in_=sc, func=AF.Exp,
                                     scale=rrk_all[:, m, h:h + 1])
                # accumulate exp tiles on GPSIMD (sums over the m-tiles)
                if m == 0:
                    nc.gpsimd.tensor_copy(out=acc, in_=at)
                else:
                    nc.gpsimd.tensor_add(out=acc, in0=acc, in1=at)
                first, last = (m == 0), (m == NT - 1)
                v_mh = vnat[:, m, h * 128:(h + 1) * 128]
                nc.tensor.matmul(o_ps[:, 0:512], v_mh, at[:, 0:512],
                                 start=first, stop=last)
                nc.tensor.matmul(o_ps[:, 512:1024], v_mh, at[:, 512:1024],
                                 start=first, stop=last)
            # sum over the sk partitions -> broadcast row, then normalize
            sums_b = rs_p.tile([128, S], F32, tag="sums_b")
            nc.gpsimd.partition_all_reduce(sums_b, acc, 128, bass_isa.ReduceOp.add)
            nc.vector.reciprocal(out=sums_b, in_=sums_b)
            nc.vector.tensor_mul(out=oT[:, h, :], in0=o_ps, in1=sums_b)

        pso_cm.__exit__(None, None, None)
        pssc_cm.__exit__(None, None, None)
        rs_cm.__exit__(None, None, None)
        at_cm.__exit__(None, None, None)
        qkT_cm.__exit__(None, None, None)
        v_cm.__exit__(None, None, None)

        # ---- phase F: out projection (transposed out)
        f_cm = tc.tile_pool(name="phF", bufs=3, side="left")
        f_p = f_cm.__enter__()
        psf_cm = tc.tile_pool(name="ps_out", bufs=2, space="PSUM")
        psf = psf_cm.__enter__()
        wo_r = wo.rearrange("(kb p) d -> p kb d", p=128)
        for m in range(KT):
            wo_t = f_p.tile([128, HG, 128], F32, tag="wo_t")
            nc.sync.dma_start(out=wo_t, in_=wo_r[:, :, m * 128:(m + 1) * 128])
            po = psf.tile([128, S], F32, tag="po")
            for kb in range(HG):
                first, last = (kb == 0), (kb == HG - 1)
                nc.tensor.matmul(po[:, 0:512], wo_t[:, kb, :], oT[:, kb, 0:512],
                                 start=first, stop=last)
                nc.tensor.matmul(po[:, 512:1024], wo_t[:, kb, :], oT[:, kb, 512:1024],
                                 start=first, stop=last)
            ot_t = f_p.tile([128, S], F32, tag="ot_t")
            nc.scalar.activation(out=ot_t, in_=po, func=AF.Identity,
                                 bias=vb_sb[:, m:m + 1], scale=gate_sb[:, m:m + 1])
            nc.sync.dma_start(out=out_t[m * 128:(m + 1) * 128, :], in_=ot_t)
        psf_cm.__exit__(None, None, None)
        f_cm.__exit__(None, None, None)
        oT_cm.__exit__(None, None, None)
        misc_cm.__exit__(None, None, None)

    nc.compile()
    return nc


_NC_CACHE = {}


def _get_nc(has_qkv_bias, has_norm_w):
    key = (has_qkv_bias, has_norm_w)
    if key not in _NC_CACHE:
        _NC_CACHE[key] = build_nc(*key)
    return _NC_CACHE[key]


def prep_in_maps(x, mod, cos, sin, qkv_w, qkv_b, mod_w, mod_b, out_w, out_b,
                 norm_q_w, norm_k_w):
    """Host-side sharding. Returns (in_maps, flags, x_np)."""
    x = np.asarray(x, dtype=np.float32)
    m3 = np.asarray(mod, np.float32) @ np.asarray(mod_w, np.float32) \
        + np.asarray(mod_b, np.float32)
    bias, scale, gatef = np.split(m3, 3, axis=-1)          # [B, D] each
    scale1p = (1.0 + scale).astype(np.float32)
    vbf = (np.asarray(out_b, np.float32)[None, :] * gatef).astype(np.float32)

    qkv_b = np.asarray(qkv_b, np.float32)
    has_qkv_bias = bool(np.any(qkv_b != 0.0))
    has_norm_w = not (np.allclose(norm_q_w, 1.0) and np.allclose(norm_k_w, 1.0))

    cosc = np.ascontiguousarray(np.asarray(cos, np.float32))
    sinc = np.ascontiguousarray(np.asarray(sin, np.float32))
    qkv_w = np.asarray(qkv_w, np.float32)
    out_w = np.asarray(out_w, np.float32)

    in_maps = []
    for c in range(N_CORES):
        b, g = divmod(c, 2)
        lo = g * GCOLS
        im = {
            "x": np.ascontiguousarray(x[b]),
            "cos": cosc, "sin": sinc,
            "wq": np.ascontiguousarray(qkv_w[:, lo:lo + GCOLS]),
            "wk": np.ascontiguousarray(qkv_w[:, 2048 + lo:2048 + lo + GCOLS]),
            "wv": np.ascontiguousarray(qkv_w[:, 4096 + lo:4096 + lo + GCOLS]),
            "wo": np.ascontiguousarray(out_w[lo:lo + GCOLS, :]),
            "scale1p": np.ascontiguousarray(scale1p[b].reshape(KT, 128).T),
            "biasm": np.ascontiguousarray(bias[b].reshape(KT, 128).T),
            "gate": np.ascontiguousarray(gatef[b].reshape(KT, 128).T),
            "vb": np.ascontiguousarray(
                (vbf[b] if g == 0 else np.zeros_like(vbf[b])).reshape(KT, 128).T),
        }
        if has_qkv_bias:
            im["bq"] = np.ascontiguousarray(qkv_b[lo:lo + GCOLS])
            im["bk"] = np.ascontiguousarray(qkv_b[2048 + lo:2048 + lo + GCOLS])
            im["bv"] = np.ascontiguousarray(qkv_b[4096 + lo:4096 + lo + GCOLS])
        if has_norm_w:
            im["wqn"] = np.ascontiguousarray(np.asarray(norm_q_w, np.float32))
            im["wkn"] = np.ascontiguousarray(np.asarray(norm_k_w, np.float32))
        in_maps.append(im)
    return in_maps, (has_qkv_bias, has_norm_w), x


def gather(results, x):
    B = x.shape[0]
    outs = []
    for b in range(B):
        p = results[2 * b]["out_t"] + results[2 * b + 1]["out_t"]   # [D, S]
        outs.append(p.T + x[b])
    return np.stack(outs).astype(np.float32)


def kernel(**inputs) -> np.ndarray:
    in_maps, flags, x = prep_in_maps(**inputs)
    nc = _get_nc(*flags)
    res = run_bass_kernel_spmd(nc, in_maps, core_ids=list(range(N_CORES)))
    return gather(res.results, x)


if __name__ == "__main__":
    import time
    t0 = time.time()
    nc = build_nc(False, False)
    print("build+compile ok in", time.time() - t0, "s")



# revision 33
# speedup vs baseline: 2.4434x; 2.4434x over previous
"""Trainium2 Bass kernel for ModalityAttention (B=4, S=1024, D=2048, H=16, HD=128, RD=64).

Sharding: 8 cores = 4 batches x 2 head-groups (8 heads each).
Each core computes, for its (batch b, head-group g):
  layernorm(x[b]) -> modulation (scale/bias precomputed on host from mod@mod_w)
  -> qkv projection for its 8 heads -> rmsnorm + rope -> attention
  -> partial out-projection (transposed layout) with gate folded in.
Host gathers: out[b] = (partial_g0 + partial_g1).T + x[b]
(residual added on host; vb = out_b*gate folded into the g0 partial on device).
"""
import os, sys

for _p in ("/opt/trn_rl_repo", "/root/.axon_site/_ro/trn_rl_repo", "/root/.axon_site"):
    if os.path.isdir(_p) and _p not in sys.path:
        sys.path.insert(0, _p)

import numpy as np
import concourse.bass as bass
import concourse.bacc as bacc
import concourse.mybir as mybir
import concourse.tile as tile
from concourse import bass_isa
from concourse.masks import make_identity
from concourse.bass_utils import run_bass_kernel_spmd

F32 = mybir.dt.float32
F32R = mybir.dt.float32r   # same bits as fp32; 4x matmul rate when free>=256
BF16 = mybir.dt.bfloat16
AF = mybir.ActivationFunctionType
S, D, HG, HD, RD = 1024, 2048, 8, 128, 64
NT = S // 128        # 8 s-tiles
KT = D // 128        # 16 d-tiles
GCOLS = HG * HD      # 1024 columns per group per projection
EPS = 1e-6
N_CORES = 8


def _bcast_from_dram(ap, parts, reps=None):
    """DRAM AP -> partition-broadcast (and optional middle-dim repeat) source AP."""
    newap = [[0, parts]]
    if reps is not None:
        newap.append([0, reps])
    newap += list(ap.ap)
    return bass.AP(tensor=ap.tensor, offset=ap.offset, ap=newap)


def build_nc(has_qkv_bias: bool, has_norm_w: bool):
    nc = bacc.Bacc("TRN2", target_bir_lowering=False, debug=False,
                   enable_asserts=True, num_devices=N_CORES)

    x = nc.dram_tensor("x", [S, D], F32, kind="ExternalInput").ap()
    cos = nc.dram_tensor("cos", [S, RD // 2], BF16, kind="ExternalInput").ap()
    sin = nc.dram_tensor("sin", [S, RD // 2], BF16, kind="ExternalInput").ap()
    wq = nc.dram_tensor("wq", [D, GCOLS], BF16, kind="ExternalInput").ap()
    wk = nc.dram_tensor("wk", [D, GCOLS], BF16, kind="ExternalInput").ap()
    wv = nc.dram_tensor("wv", [D, GCOLS], BF16, kind="ExternalInput").ap()
    wo = nc.dram_tensor("wo", [GCOLS, D], BF16, kind="ExternalInput").ap()
    # modulation vectors, pre-reshaped on host to [128, KT] (column k = d-tile k)
    scale1p = nc.dram_tensor("scale1p", [128, KT], F32, kind="ExternalInput").ap()
    biasm = nc.dram_tensor("biasm", [128, KT], F32, kind="ExternalInput").ap()
    gate = nc.dram_tensor("gate", [128, KT], F32, kind="ExternalInput").ap()
    vb = nc.dram_tensor("vb", [128, KT], F32, kind="ExternalInput").ap()
    if has_qkv_bias:
        bq = nc.dram_tensor("bq", [GCOLS], BF16, kind="ExternalInput").ap()
        bk = nc.dram_tensor("bk", [GCOLS], BF16, kind="ExternalInput").ap()
        bv = nc.dram_tensor("bv", [GCOLS], BF16, kind="ExternalInput").ap()
    if has_norm_w:
        wqn = nc.dram_tensor("wqn", [HD], BF16, kind="ExternalInput").ap()
        wkn = nc.dram_tensor("wkn", [HD], BF16, kind="ExternalInput").ap()
    out_t = nc.dram_tensor("out_t", [D, S], F32, kind="ExternalOutput").ap()

    with tile.TileContext(nc) as tc:
        # ======== LEFT stack bottom: small persistent constants ====================
        misc_cm = tc.tile_pool(name="misc", bufs=1, side="left")
        misc = misc_cm.__enter__()
        ident = misc.tile([128, 128], BF16)
        make_identity(nc, ident)
        ones_col = misc.tile([128, 1], F32)
        nc.vector.memset(ones_col, 1.0)
        eps_t = misc.tile([128, 1], F32)
        nc.vector.memset(eps_t, EPS)
        eps128_t = misc.tile([128, 1], F32)
        nc.vector.memset(eps128_t, HD * EPS)
        gate_sb = misc.tile([128, KT], F32)
        vb_sb = misc.tile([128, KT], F32)
        rrk_all = misc.tile([128, NT, HG], F32)   # scaled k-rms reciprocals
        if has_norm_w:
            wqn_b = misc.tile([128, HG, HD], BF16)
            wkn_b = misc.tile([128, HG, HD], BF16)
        cs_tiles = []
        for m in range(NT):
            ct = misc.tile([128, RD // 2], BF16, tag=f"cos_{m}", name=f"cos_{m}")
            st = misc.tile([128, RD // 2], BF16, tag=f"sin_{m}", name=f"sin_{m}")
            cs_tiles.append((ct, st))
        # (misc DMAs are emitted after phase A so the x-tile loads go first
        #  in the HWDGE queue; these tiles are only consumed in later phases)

        # ======== RIGHT stack: big natural-layout tensors (B..E lifetimes) =========
        v_cm = tc.tile_pool(name="vpool", bufs=1, side="right")
        v_p = v_cm.__enter__()
        vnat = v_p.tile([128, NT, GCOLS], BF16)
        natqk_cm = tc.tile_pool(name="natqk", bufs=1, side="right")
        natqk = natqk_cm.__enter__()
        qnat = natqk.tile([128, NT, GCOLS], BF16)
        knat = natqk.tile([128, NT, GCOLS], BF16)

        # ======== phase A: layernorm + modulation + transpose -> xnT ===============
        xnT_cm = tc.tile_pool(name="xnT", bufs=1, side="left")
        xnT_p = xnT_cm.__enter__()
        xnT = xnT_p.tile([128, KT, S], BF16)  # [d_in_tile, d_tile, s]

        avec_cm = tc.tile_pool(name="phA_vec", bufs=1, side="left")
        avec = avec_cm.__enter__()
        s1pc = avec.tile([128, KT], F32)
        bmc = avec.tile([128, KT], F32)
        if has_qkv_bias:
            bq_b = avec.tile([128, GCOLS], BF16)
            nc.sync.dma_start(out=bq_b, in_=_bcast_from_dram(bq, 128))
            bk_b = avec.tile([128, GCOLS], BF16)
            nc.sync.dma_start(out=bk_b, in_=_bcast_from_dram(bk, 128))
            bv_b = avec.tile([128, GCOLS], BF16)
            nc.sync.dma_start(out=bv_b, in_=_bcast_from_dram(bv, 128))

        a_cm = tc.tile_pool(name="phA", bufs=3, side="left")
        a_p = a_cm.__enter__()
        a_small_cm = tc.tile_pool(name="phA_small", bufs=4, side="left")
        a_small = a_small_cm.__enter__()
        pst_cm = tc.tile_pool(name="ps_tr", bufs=4, space="PSUM")
        pst = pst_cm.__enter__()

        for i in range(NT):
            xt = a_p.tile([128, D], F32, tag="xt")
            nc.sync.dma_start(out=xt, in_=x[i * 128:(i + 1) * 128, :])
            if i == 0:
                nc.sync.dma_start(out=s1pc, in_=scale1p)
                nc.sync.dma_start(out=bmc, in_=biasm)
            stats = a_small.tile([128, 4, 6], F32, tag="stats")
            xv = xt.rearrange("p (c f) -> p c f", c=4)
            for c in range(4):
                nc.vector.bn_stats(out=stats[:, c, :], in_=xv[:, c, :])
            mv = a_small.tile([128, 2], F32, tag="mv")
            nc.vector.bn_aggr(out=mv, in_=stats)
            rstd = a_small.tile([128, 1], F32, tag="rstd")
            nc.scalar.activation(out=rstd, in_=mv[:, 1:2], func=AF.Sqrt,
                                 bias=eps_t, scale=1.0)
            nc.vector.reciprocal(out=rstd, in_=rstd)
            nmr = a_small.tile([128, 1], F32, tag="nmr")
            nc.vector.tensor_mul(out=nmr, in0=mv[:, 0:1], in1=rstd)
            nc.scalar.mul(out=nmr, in_=nmr, mul=-1.0)
            xtb = a_p.tile([128, D], BF16, tag="xtb")
            nc.scalar.activation(out=xtb, in_=xt, func=AF.Identity,
                                 bias=nmr, scale=rstd)
            for k in range(KT):
                pt = pst.tile([128, 128], BF16, tag="pt")
                nc.tensor.transpose(pt, xtb[:, k * 128:(k + 1) * 128], ident)
                # modulation fused into the evac: xnT = pt * (1+scale[d]) + bias[d]
                nc.scalar.activation(out=xnT[:, k, i * 128:(i + 1) * 128], in_=pt,
                                     func=AF.Identity,
                                     bias=bmc[:, k:k + 1], scale=s1pc[:, k:k + 1])

        # deferred misc loads (consumed in phases C/E/F)
        nc.sync.dma_start(out=gate_sb, in_=gate)
        nc.sync.dma_start(out=vb_sb, in_=vb)
        if has_norm_w:
            nc.sync.dma_start(out=wqn_b, in_=_bcast_from_dram(wqn, 128, reps=HG))
            nc.sync.dma_start(out=wkn_b, in_=_bcast_from_dram(wkn, 128, reps=HG))
        for m in range(NT):
            ct, st = cs_tiles[m]
            nc.sync.dma_start(out=ct, in_=cos[m * 128:(m + 1) * 128, :])
            nc.sync.dma_start(out=st, in_=sin[m * 128:(m + 1) * 128, :])

        pst_cm.__exit__(None, None, None)
        a_small_cm.__exit__(None, None, None)
        a_cm.__exit__(None, None, None)

        # phase C pools opened BEFORE phase B emission so the rms/rope work can
        # overlap the tail of the qkv matmuls (no pool-boundary serialization).
        # With qkv biases present SBUF is too tight for the overlap; in that
        # case C pools open after B instead.
        overlap_c = not has_qkv_bias
        if overlap_c:
            c_cm = tc.tile_pool(name="phC", bufs=2, side="left")
            c_p = c_cm.__enter__()
            c_small_cm = tc.tile_pool(name="phC_small", bufs=2, side="left")
            c_small = c_small_cm.__enter__()

        # ======== phase B: qkv projections (natural layout) ========================
        w_cm = tc.tile_pool(name="wstream", bufs=3, side="right")
        w_p = w_cm.__enter__()
        psb_cm = tc.tile_pool(name="ps_qkv", bufs=1, space="PSUM")
        psb = psb_cm.__enter__()

        for (wdram, nat) in ((wq, qnat), (wk, knat), (wv, vnat)):
            for n in range(2):
                ps = [psb.tile([128, 512], F32, tag=f"ps{m}", name=f"ps{m}")
                      for m in range(NT)]
                for k in range(KT):
                    wt = w_p.tile([128, 512], BF16, tag="wt")
                    nc.sync.dma_start(
                        out=wt, in_=wdram[k * 128:(k + 1) * 128, n * 512:(n + 1) * 512])
                    for m in range(NT):
                        nc.tensor.matmul(ps[m],
                                         xnT[:, k, m * 128:(m + 1) * 128], wt,
                                         start=(k == 0), stop=(k == KT - 1))
                for m in range(NT):
                    nc.scalar.copy(out=nat[:, m, n * 512:(n + 1) * 512], in_=ps[m])
        if has_qkv_bias:
            for m in range(NT):
                nc.gpsimd.tensor_add(out=qnat[:, m, :], in0=qnat[:, m, :], in1=bq_b)
                nc.gpsimd.tensor_add(out=knat[:, m, :], in0=knat[:, m, :], in1=bk_b)
                nc.gpsimd.tensor_add(out=vnat[:, m, :], in0=vnat[:, m, :], in1=bv_b)

        psb_cm.__exit__(None, None, None)
        w_cm.__exit__(None, None, None)

        # ======== phase C: rmsnorm + rope on q, k (natural, in place) ==============
        if not overlap_c:
            c_cm = tc.tile_pool(name="phC", bufs=2, side="left")
            c_p = c_cm.__enter__()
            c_small_cm = tc.tile_pool(name="phC_small", bufs=2, side="left")
            c_small = c_small_cm.__enter__()

        for m in range(NT):
            qm = qnat[:, m, :]
            km = knat[:, m, :]
            (ct, st) = cs_tiles[m]
            cb = ct.unsqueeze(1).broadcast_to([128, HG, RD // 2])
            sb_ = st.unsqueeze(1).broadcast_to([128, HG, RD // 2])

            # rms stats (on raw q/k, before norm-w and rope)
            sq = c_p.tile([128, GCOLS], BF16, tag="sqk")
            nc.vector.tensor_mul(out=sq, in0=qm, in1=qm)
            ssq = c_small.tile([128, HG], F32, tag="ssq")
            nc.vector.reduce_sum(out=ssq, in_=sq.rearrange("p (h d) -> p h d", h=HG),
                                 axis=mybir.AxisListType.X)
            rrq = c_small.tile([128, HG], F32, tag="rrq")
            nc.scalar.activation(out=rrq, in_=ssq, func=AF.Sqrt,
                                 bias=eps_t, scale=1.0 / HD)
            nc.vector.reciprocal(out=rrq, in_=rrq)

            sk_ = c_p.tile([128, GCOLS], BF16, tag="sqk")
            nc.vector.tensor_mul(out=sk_, in0=km, in1=km)
            ssk = c_small.tile([128, HG], F32, tag="ssk")
            nc.vector.reduce_sum(out=ssk, in_=sk_.rearrange("p (h d) -> p h d", h=HG),
                                 axis=mybir.AxisListType.X)
            nc.scalar.activation(out=rrk_all[:, m, :], in_=ssk, func=AF.Sqrt,
                                 bias=eps128_t, scale=1.0)
            nc.vector.reciprocal(out=rrk_all[:, m, :], in_=rrk_all[:, m, :])

            if has_norm_w:
                nc.vector.tensor_mul(out=qm.rearrange("p (h d) -> p h d", h=HG),
                                     in0=qm.rearrange("p (h d) -> p h d", h=HG),
                                     in1=wqn_b)
                nc.vector.tensor_mul(out=km.rearrange("p (h d) -> p h d", h=HG),
                                     in0=km.rearrange("p (h d) -> p h d", h=HG),
                                     in1=wkn_b)

            for mm in (qm, km):
                mv_ = mm.rearrange("p (h i two) -> p h i two", h=HG, two=2)
                x0 = mv_[:, :, 0:RD // 2, 0]
                x1 = mv_[:, :, 0:RD // 2, 1]
                t0 = c_small.tile([128, HG, RD // 2], BF16, tag="t0")
                t1 = c_small.tile([128, HG, RD // 2], BF16, tag="t1")
                t2 = c_small.tile([128, HG, RD // 2], BF16, tag="t2")
                t3 = c_small.tile([128, HG, RD // 2], BF16, tag="t3")
                nc.vector.tensor_mul(out=t0, in0=x0, in1=cb)
                nc.vector.tensor_mul(out=t1, in0=x1, in1=sb_)
                nc.vector.tensor_mul(out=t2, in0=x0, in1=sb_)
                nc.vector.tensor_mul(out=t3, in0=x1, in1=cb)
                nc.gpsimd.tensor_sub(out=x0, in0=t0, in1=t1)
                nc.gpsimd.tensor_add(out=x1, in0=t2, in1=t3)

            # apply q rms reciprocal (k's is folded into the exp scale later)
            rrq_b = rrq.unsqueeze(2).broadcast_to([128, HG, HD])
            nc.vector.tensor_mul(out=qm.rearrange("p (h d) -> p h d", h=HG),
                                 in0=qm.rearrange("p (h d) -> p h d", h=HG),
                                 in1=rrq_b)

        c_small_cm.__exit__(None, None, None)
        c_cm.__exit__(None, None, None)
        avec_cm.__exit__(None, None, None)
        xnT_cm.__exit__(None, None, None)

        # ======== phases D/E/F share the left stack: oT under qkT ==================
        oT_cm = tc.tile_pool(name="oT", bufs=1, side="left")
        oT_p = oT_cm.__enter__()
        oT = oT_p.tile([128, HG, S], BF16)

        # ---- phase D: transpose q, k -> [hd, s] per head
        qkT_cm = tc.tile_pool(name="qkT", bufs=1, side="left")
        qkT_p = qkT_cm.__enter__()
        qT = qkT_p.tile([128, HG, S], BF16)
        kT = qkT_p.tile([128, HG, S], BF16)
        pst2_cm = tc.tile_pool(name="ps_tr2", bufs=4, space="PSUM")
        pst2 = pst2_cm.__enter__()
        for (nat, dst) in ((qnat, qT), (knat, kT)):
            for h in range(HG):
                for m in range(NT):
                    pt2 = pst2.tile([128, 128], BF16, tag="pt2")
                    nc.tensor.transpose(pt2, nat[:, m, h * 128:(h + 1) * 128], ident)
                    nc.scalar.copy(out=dst[:, h, m * 128:(m + 1) * 128], in_=pt2)
        pst2_cm.__exit__(None, None, None)
        natqk_cm.__exit__(None, None, None)

        # ---- phase E: attention per head
        at_cm = tc.tile_pool(name="attn", bufs=3, side="left")
        at_p = at_cm.__enter__()
        rs_cm = tc.tile_pool(name="rsb", bufs=2, side="left")
        rs_p = rs_cm.__enter__()
        pssc_cm = tc.tile_pool(name="ps_sc", bufs=3, space="PSUM")
        pssc = pssc_cm.__enter__()
        pso_cm = tc.tile_pool(name="ps_o", bufs=1, space="PSUM")
        pso = pso_cm.__enter__()

        for h in range(HG):
            o_ps = pso.tile([128, S], F32, tag="o_ps")
            acc = rs_p.tile([128, S], BF16, tag="acc")
            for m in range(NT):
                sc = pssc.tile([128, S], F32, tag="sc")
                lhs_k = kT[:, h, m * 128:(m + 1) * 128]
                nc.tensor.matmul(sc[:, 0:512], lhs_k, qT[:, h, 0:512],
                                 start=True, stop=True)
                nc.tensor.matmul(sc[:, 512:1024], lhs_k, qT[:, h, 512:1024],
                                 start=True, stop=True)
                at = at_p.tile([128, S], BF16, tag="at", name="at")
                nc.scalar.activation(out=at, in_=sc, func=AF.Exp,
                                     scale=rrk_all[:, m, h:h + 1])
                # accumulate exp tiles on GPSIMD (sums over the m-tiles)
                if m == 0:
                    nc.gpsimd.tensor_copy(out=acc, in_=at)
                else:
                    nc.gpsimd.tensor_add(out=acc, in0=acc, in1=at)
                first, last = (m == 0), (m == NT - 1)
                v_mh = vnat[:, m, h * 128:(h + 1) * 128]
                nc.tensor.matmul(o_ps[:, 0:512], v_mh, at[:, 0:512],
                                 start=first, stop=last)
                nc.tensor.matmul(o_ps[:, 512:1024], v_mh, at[:, 512:1024],
                                 start=first, stop=last)
            # sum over the sk partitions -> broadcast row, then normalize
            sums_b = rs_p.tile([128, S], F32, tag="sums_b")
            nc.gpsimd.partition_all_reduce(sums_b, acc, 128, bass_isa.ReduceOp.add)
            nc.vector.reciprocal(out=sums_b, in_=sums_b)
            nc.vector.tensor_mul(out=oT[:, h, :], in0=o_ps, in1=sums_b)

        pso_cm.__exit__(None, None, None)
        pssc_cm.__exit__(None, None, None)
        rs_cm.__exit__(None, None, None)
        at_cm.__exit__(None, None, None)
        qkT_cm.__exit__(None, None, None)
        v_cm.__exit__(None, None, None)

        # ---- phase F: out projection (transposed out)
        f_cm = tc.tile_pool(name="phF", bufs=3, side="left")
        f_p = f_cm.__enter__()
        psf_cm = tc.tile_pool(name="ps_out", bufs=2, space="PSUM")
        psf = psf_cm.__enter__()
        wo_r = wo.rearrange("(kb p) d -> p kb d", p=128)
        for m in range(KT):
            wo_t = f_p.tile([128, HG, 128], BF16, tag="wo_t")
            nc.sync.dma_start(out=wo_t, in_=wo_r[:, :, m * 128:(m + 1) * 128])
            po = psf.tile([128, S], F32, tag="po")
            for kb in range(HG):
                first, last = (kb == 0), (kb == HG - 1)
                nc.tensor.matmul(po[:, 0:512], wo_t[:, kb, :], oT[:, kb, 0:512],
                                 start=first, stop=last)
                nc.tensor.matmul(po[:, 512:1024], wo_t[:, kb, :],
                                 oT[:, kb, 512:1024],
                                 start=first, stop=last)
            ot_t = f_p.tile([128, S], F32, tag="ot_t")
            nc.scalar.activation(out=ot_t, in_=po, func=AF.Identity,
                                 bias=vb_sb[:, m:m + 1], scale=gate_sb[:, m:m + 1])
            nc.sync.dma_start(out=out_t[m * 128:(m + 1) * 128, :], in_=ot_t)
        psf_cm.__exit__(None, None, None)
        f_cm.__exit__(None, None, None)
        oT_cm.__exit__(None, None, None)
        misc_cm.__exit__(None, None, None)

    nc.compile()
    return nc


_NC_CACHE = {}


def _get_nc(has_qkv_bias, has_norm_w):
    key = (has_qkv_bias, has_norm_w)
    if key not in _NC_CACHE:
        _NC_CACHE[key] = build_nc(*key)
    return _NC_CACHE[key]


def prep_in_maps(x, mod, cos, sin, qkv_w, qkv_b, mod_w, mod_b, out_w, out_b,
                 norm_q_w, norm_k_w):
    """Host-side sharding. Returns (in_maps, flags, x_np)."""
    x = np.asarray(x, dtype=np.float32)
    m3 = np.asarray(mod, np.float32) @ np.asarray(mod_w, np.float32) \
        + np.asarray(mod_b, np.float32)
    bias, scale, gatef = np.split(m3, 3, axis=-1)          # [B, D] each
    scale1p = (1.0 + scale).astype(np.float32)
    vbf = (np.asarray(out_b, np.float32)[None, :] * gatef).astype(np.float32)

    qkv_b = np.asarray(qkv_b, np.float32)
    has_qkv_bias = bool(np.any(qkv_b != 0.0))
    has_norm_w = not (np.allclose(norm_q_w, 1.0) and np.allclose(norm_k_w, 1.0))

    import ml_dtypes
    bf16 = ml_dtypes.bfloat16
    cosc = np.ascontiguousarray(np.asarray(cos, np.float32).astype(bf16))
    sinc = np.ascontiguousarray(np.asarray(sin, np.float32).astype(bf16))
    qkv_w = np.asarray(qkv_w, np.float32).astype(bf16)
    out_w = np.asarray(out_w, np.float32).astype(bf16)

    in_maps = []
    for c in range(N_CORES):
        b, g = divmod(c, 2)
        lo = g * GCOLS
        im = {
            "x": np.ascontiguousarray(x[b]),
            "cos": cosc, "sin": sinc,
            "wq": np.ascontiguousarray(qkv_w[:, lo:lo + GCOLS]),
            "wk": np.ascontiguousarray(qkv_w[:, 2048 + lo:2048 + lo + GCOLS]),
            "wv": np.ascontiguousarray(qkv_w[:, 4096 + lo:4096 + lo + GCOLS]),
            "wo": np.ascontiguousarray(out_w[lo:lo + GCOLS, :]),
            "scale1p": np.ascontiguousarray(scale1p[b].reshape(KT, 128).T),
            "biasm": np.ascontiguousarray(bias[b].reshape(KT, 128).T),
            "gate": np.ascontiguousarray(gatef[b].reshape(KT, 128).T),
            "vb": np.ascontiguousarray(
                (vbf[b] if g == 0 else np.zeros_like(vbf[b])).reshape(KT, 128).T),
        }
        if has_qkv_bias:
            im["bq"] = np.ascontiguousarray(qkv_b[lo:lo + GCOLS].astype(bf16))
            im["bk"] = np.ascontiguousarray(
                qkv_b[2048 + lo:2048 + lo + GCOLS].astype(bf16))
            im["bv"] = np.ascontiguousarray(
                qkv_b[4096 + lo:4096 + lo + GCOLS].astype(bf16))
        if has_norm_w:
            im["wqn"] = np.ascontiguousarray(
                np.asarray(norm_q_w, np.float32).astype(bf16))
            im["wkn"] = np.ascontiguousarray(
                np.asarray(norm_k_w, np.float32).astype(bf16))
        in_maps.append(im)
    return in_maps, (has_qkv_bias, has_norm_w), x


def gather(results, x):
    B = x.shape[0]
    outs = []
    for b in range(B):
        p = results[2 * b]["out_t"] + results[2 * b + 1]["out_t"]   # [D, S]
        outs.append(p.T + x[b])
    return np.stack(outs).astype(np.float32)


def kernel(**inputs) -> np.ndarray:
    in_maps, flags, x = prep_in_maps(**inputs)
    nc = _get_nc(*flags)
    res = run_bass_kernel_spmd(nc, in_maps, core_ids=list(range(N_CORES)))
    return gather(res.results, x)


if __name__ == "__main__":
    import time
    t0 = time.time()
    nc = build_nc(False, False)
    print("build+compile ok in", time.time() - t0, "s")



# revision 36
# speedup vs baseline: 3.0263x; 1.2386x over previous
"""Trainium2 Bass kernel for ModalityAttention (B=4, S=1024, D=2048, H=16, HD=128, RD=64).

Sharding: 8 cores = 4 batches x 2 head-groups (8 heads each).
Each core computes, for its (batch b, head-group g):
  layernorm(x[b]) -> modulation (scale/bias precomputed on host from mod@mod_w)
  -> qkv projection for its 8 heads -> rmsnorm + rope -> attention
  -> partial out-projection (transposed layout) with gate folded in.
Host gathers: out[b] = (partial_g0 + partial_g1).T + x[b]
(residual added on host; vb = out_b*gate folded into the g0 partial on device).
"""
import os, sys

for _p in ("/opt/trn_rl_repo", "/root/.axon_site/_ro/trn_rl_repo", "/root/.axon_site"):
    if os.path.isdir(_p) and _p not in sys.path:
        sys.path.insert(0, _p)

import numpy as np
import concourse.bass as bass
import concourse.bacc as bacc
import concourse.mybir as mybir
import concourse.tile as tile
from concourse import bass_isa
from concourse.masks import make_identity
from concourse.bass_utils import run_bass_kernel_spmd

F32 = mybir.dt.float32
F32R = mybir.dt.float32r   # same bits as fp32; 4x matmul rate when free>=256
BF16 = mybir.dt.bfloat16
AF = mybir.ActivationFunctionType
S, D, HG, HD, RD = 1024, 2048, 8, 128, 64
NT = S // 128        # 8 s-tiles
KT = D // 128        # 16 d-tiles
GCOLS = HG * HD      # 1024 columns per group per projection
EPS = 1e-6
N_CORES = 8


def _bcast_from_dram(ap, parts, reps=None):
    """DRAM AP -> partition-broadcast (and optional middle-dim repeat) source AP."""
    newap = [[0, parts]]
    if reps is not None:
        newap.append([0, reps])
    newap += list(ap.ap)
    return bass.AP(tensor=ap.tensor, offset=ap.offset, ap=newap)


def build_nc(has_qkv_bias: bool, has_norm_w: bool):
    nc = bacc.Bacc("TRN2", target_bir_lowering=False, debug=False,
                   enable_asserts=True, num_devices=N_CORES)

    x = nc.dram_tensor("x", [S, D], F32, kind="ExternalInput").ap()
    cos = nc.dram_tensor("cos", [S, RD // 2], BF16, kind="ExternalInput").ap()
    sin = nc.dram_tensor("sin", [S, RD // 2], BF16, kind="ExternalInput").ap()
    wq = nc.dram_tensor("wq", [D, GCOLS], BF16, kind="ExternalInput").ap()
    wk = nc.dram_tensor("wk", [D, GCOLS], BF16, kind="ExternalInput").ap()
    wv = nc.dram_tensor("wv", [D, GCOLS], BF16, kind="ExternalInput").ap()
    wo = nc.dram_tensor("wo", [GCOLS, D], BF16, kind="ExternalInput").ap()
    # modulation vectors, pre-reshaped on host to [128, KT] (column k = d-tile k)
    scale1p = nc.dram_tensor("scale1p", [128, KT], F32, kind="ExternalInput").ap()
    biasm = nc.dram_tensor("biasm", [128, KT], F32, kind="ExternalInput").ap()
    gate = nc.dram_tensor("gate", [128, KT], F32, kind="ExternalInput").ap()
    vb = nc.dram_tensor("vb", [128, KT], F32, kind="ExternalInput").ap()
    if has_qkv_bias:
        bq = nc.dram_tensor("bq", [GCOLS], BF16, kind="ExternalInput").ap()
        bk = nc.dram_tensor("bk", [GCOLS], BF16, kind="ExternalInput").ap()
        bv = nc.dram_tensor("bv", [GCOLS], BF16, kind="ExternalInput").ap()
    if has_norm_w:
        wqn = nc.dram_tensor("wqn", [HD], BF16, kind="ExternalInput").ap()
        wkn = nc.dram_tensor("wkn", [HD], BF16, kind="ExternalInput").ap()
    out_t = nc.dram_tensor("out_t", [D, S], F32, kind="ExternalOutput").ap()

    with tile.TileContext(nc) as tc:
        # ======== LEFT stack bottom: small persistent constants ====================
        misc_cm = tc.tile_pool(name="misc", bufs=1, side="left")
        misc = misc_cm.__enter__()
        ident = misc.tile([128, 128], BF16)
        make_identity(nc, ident)
        ones_col = misc.tile([128, 1], F32)
        nc.vector.memset(ones_col, 1.0)
        eps_t = misc.tile([128, 1], F32)
        nc.vector.memset(eps_t, EPS)
        eps128_t = misc.tile([128, 1], F32)
        nc.vector.memset(eps128_t, HD * EPS)
        gate_sb = misc.tile([128, KT], F32)
        vb_sb = misc.tile([128, KT], F32)
        rrk_all = misc.tile([128, NT, HG], F32)   # scaled k-rms reciprocals
        if has_norm_w:
            wqn_b = misc.tile([128, HG, HD], BF16)
            wkn_b = misc.tile([128, HG, HD], BF16)
        cs_tiles = []
        for m in range(NT):
            ct = misc.tile([128, RD // 2], BF16, tag=f"cos_{m}", name=f"cos_{m}")
            st = misc.tile([128, RD // 2], BF16, tag=f"sin_{m}", name=f"sin_{m}")
            cs_tiles.append((ct, st))
        # (misc DMAs are emitted after phase A so the x-tile loads go first
        #  in the HWDGE queue; these tiles are only consumed in later phases)

        # ======== RIGHT stack: big natural-layout tensors (B..E lifetimes) =========
        v_cm = tc.tile_pool(name="vpool", bufs=1, side="right")
        v_p = v_cm.__enter__()
        vnat = v_p.tile([128, NT, GCOLS], BF16)
        natqk_cm = tc.tile_pool(name="natqk", bufs=1, side="right")
        natqk = natqk_cm.__enter__()
        qnat = natqk.tile([128, NT, GCOLS], BF16)
        knat = natqk.tile([128, NT, GCOLS], BF16)

        # ======== phase A: layernorm + modulation + transpose -> xnT ===============
        xnT_cm = tc.tile_pool(name="xnT", bufs=1, side="left")
        xnT_p = xnT_cm.__enter__()
        xnT = xnT_p.tile([128, KT, S], BF16)  # [d_in_tile, d_tile, s]

        avec_cm = tc.tile_pool(name="phA_vec", bufs=1, side="left")
        avec = avec_cm.__enter__()
        s1pc = avec.tile([128, KT], F32)
        bmc = avec.tile([128, KT], F32)
        if has_qkv_bias:
            bq_b = avec.tile([128, GCOLS], BF16)
            nc.sync.dma_start(out=bq_b, in_=_bcast_from_dram(bq, 128))
            bk_b = avec.tile([128, GCOLS], BF16)
            nc.sync.dma_start(out=bk_b, in_=_bcast_from_dram(bk, 128))
            bv_b = avec.tile([128, GCOLS], BF16)
            nc.sync.dma_start(out=bv_b, in_=_bcast_from_dram(bv, 128))

        a_cm = tc.tile_pool(name="phA", bufs=3, side="left")
        a_p = a_cm.__enter__()
        a_small_cm = tc.tile_pool(name="phA_small", bufs=4, side="left")
        a_small = a_small_cm.__enter__()
        pst_cm = tc.tile_pool(name="ps_tr", bufs=4, space="PSUM")
        pst = pst_cm.__enter__()

        for i in range(NT):
            xt = a_p.tile([128, D], F32, tag="xt")
            nc.sync.dma_start(out=xt, in_=x[i * 128:(i + 1) * 128, :])
            if i == 0:
                nc.sync.dma_start(out=s1pc, in_=scale1p)
                nc.sync.dma_start(out=bmc, in_=biasm)
            stats = a_small.tile([128, 4, 6], F32, tag="stats")
            xv = xt.rearrange("p (c f) -> p c f", c=4)
            for c in range(4):
                nc.vector.bn_stats(out=stats[:, c, :], in_=xv[:, c, :])
            mv = a_small.tile([128, 2], F32, tag="mv")
            nc.vector.bn_aggr(out=mv, in_=stats)
            rstd = a_small.tile([128, 1], F32, tag="rstd")
            nc.scalar.activation(out=rstd, in_=mv[:, 1:2], func=AF.Sqrt,
                                 bias=eps_t, scale=1.0)
            nc.vector.reciprocal(out=rstd, in_=rstd)
            nmr = a_small.tile([128, 1], F32, tag="nmr")
            nc.vector.tensor_mul(out=nmr, in0=mv[:, 0:1], in1=rstd)
            nc.scalar.mul(out=nmr, in_=nmr, mul=-1.0)
            xtb = a_p.tile([128, D], BF16, tag="xtb")
            nc.scalar.activation(out=xtb, in_=xt, func=AF.Identity,
                                 bias=nmr, scale=rstd)
            for k in range(KT):
                pt = pst.tile([128, 128], BF16, tag="pt")
                nc.tensor.transpose(pt, xtb[:, k * 128:(k + 1) * 128], ident)
                # modulation fused into the evac: xnT = pt * (1+scale[d]) + bias[d]
                # alternate ACT / DVE so neither engine serializes phase A
                if k % 2 == 0:
                    nc.scalar.activation(out=xnT[:, k, i * 128:(i + 1) * 128],
                                         in_=pt, func=AF.Identity,
                                         bias=bmc[:, k:k + 1], scale=s1pc[:, k:k + 1])
                else:
                    nc.vector.tensor_scalar(
                        out=xnT[:, k, i * 128:(i + 1) * 128], in0=pt,
                        scalar1=s1pc[:, k:k + 1], scalar2=bmc[:, k:k + 1],
                        op0=mybir.AluOpType.mult, op1=mybir.AluOpType.add)

        # deferred misc loads (consumed in phases C/E/F)
        nc.sync.dma_start(out=gate_sb, in_=gate)
        nc.sync.dma_start(out=vb_sb, in_=vb)
        if has_norm_w:
            nc.sync.dma_start(out=wqn_b, in_=_bcast_from_dram(wqn, 128, reps=HG))
            nc.sync.dma_start(out=wkn_b, in_=_bcast_from_dram(wkn, 128, reps=HG))
        for m in range(NT):
            ct, st = cs_tiles[m]
            nc.sync.dma_start(out=ct, in_=cos[m * 128:(m + 1) * 128, :])
            nc.sync.dma_start(out=st, in_=sin[m * 128:(m + 1) * 128, :])

        pst_cm.__exit__(None, None, None)
        a_small_cm.__exit__(None, None, None)
        a_cm.__exit__(None, None, None)

        # phase C pools opened BEFORE phase B emission so the rms/rope work can
        # overlap the tail of the qkv matmuls (no pool-boundary serialization).
        # With qkv biases present SBUF is too tight for the overlap; in that
        # case C pools open after B instead.
        overlap_c = not has_qkv_bias
        if overlap_c:
            c_cm = tc.tile_pool(name="phC", bufs=2, side="left")
            c_p = c_cm.__enter__()
            c_small_cm = tc.tile_pool(name="phC_small", bufs=2, side="left")
            c_small = c_small_cm.__enter__()

        # ======== phase B: qkv projections (natural layout) ========================
        w_cm = tc.tile_pool(name="wstream", bufs=3, side="right")
        w_p = w_cm.__enter__()
        psb_cm = tc.tile_pool(name="ps_qkv", bufs=1, space="PSUM")
        psb = psb_cm.__enter__()

        for (wdram, nat) in ((wq, qnat), (wk, knat), (wv, vnat)):
            for n in range(2):
                ps = [psb.tile([128, 512], F32, tag=f"ps{m}", name=f"ps{m}")
                      for m in range(NT)]
                for k in range(KT):
                    wt = w_p.tile([128, 512], BF16, tag="wt")
                    nc.sync.dma_start(
                        out=wt, in_=wdram[k * 128:(k + 1) * 128, n * 512:(n + 1) * 512])
                    for m in range(NT):
                        nc.tensor.matmul(ps[m],
                                         xnT[:, k, m * 128:(m + 1) * 128], wt,
                                         start=(k == 0), stop=(k == KT - 1))
                for m in range(NT):
                    nc.scalar.copy(out=nat[:, m, n * 512:(n + 1) * 512], in_=ps[m])
        if has_qkv_bias:
            for m in range(NT):
                nc.gpsimd.tensor_add(out=qnat[:, m, :], in0=qnat[:, m, :], in1=bq_b)
                nc.gpsimd.tensor_add(out=knat[:, m, :], in0=knat[:, m, :], in1=bk_b)
                nc.gpsimd.tensor_add(out=vnat[:, m, :], in0=vnat[:, m, :], in1=bv_b)

        psb_cm.__exit__(None, None, None)
        w_cm.__exit__(None, None, None)

        # ======== phase C: rmsnorm + rope on q, k (natural, in place) ==============
        if not overlap_c:
            c_cm = tc.tile_pool(name="phC", bufs=2, side="left")
            c_p = c_cm.__enter__()
            c_small_cm = tc.tile_pool(name="phC_small", bufs=2, side="left")
            c_small = c_small_cm.__enter__()

        for m in range(NT):
            qm = qnat[:, m, :]
            km = knat[:, m, :]
            (ct, st) = cs_tiles[m]
            cb = ct.unsqueeze(1).broadcast_to([128, HG, RD // 2])
            sb_ = st.unsqueeze(1).broadcast_to([128, HG, RD // 2])

            # rms stats (on raw q/k, before norm-w and rope)
            sq = c_p.tile([128, GCOLS], BF16, tag="sqk")
            nc.vector.tensor_mul(out=sq, in0=qm, in1=qm)
            ssq = c_small.tile([128, HG], F32, tag="ssq")
            nc.vector.reduce_sum(out=ssq, in_=sq.rearrange("p (h d) -> p h d", h=HG),
                                 axis=mybir.AxisListType.X)
            rrq = c_small.tile([128, HG], F32, tag="rrq")
            nc.scalar.activation(out=rrq, in_=ssq, func=AF.Sqrt,
                                 bias=eps_t, scale=1.0 / HD)
            nc.vector.reciprocal(out=rrq, in_=rrq)

            sk_ = c_p.tile([128, GCOLS], BF16, tag="sqk")
            nc.vector.tensor_mul(out=sk_, in0=km, in1=km)
            ssk = c_small.tile([128, HG], F32, tag="ssk")
            nc.vector.reduce_sum(out=ssk, in_=sk_.rearrange("p (h d) -> p h d", h=HG),
                                 axis=mybir.AxisListType.X)
            nc.scalar.activation(out=rrk_all[:, m, :], in_=ssk, func=AF.Sqrt,
                                 bias=eps128_t, scale=1.0)
            nc.vector.reciprocal(out=rrk_all[:, m, :], in_=rrk_all[:, m, :])

            if has_norm_w:
                nc.vector.tensor_mul(out=qm.rearrange("p (h d) -> p h d", h=HG),
                                     in0=qm.rearrange("p (h d) -> p h d", h=HG),
                                     in1=wqn_b)
                nc.vector.tensor_mul(out=km.rearrange("p (h d) -> p h d", h=HG),
                                     in0=km.rearrange("p (h d) -> p h d", h=HG),
                                     in1=wkn_b)

            for mm in (qm, km):
                mv_ = mm.rearrange("p (h i two) -> p h i two", h=HG, two=2)
                x0 = mv_[:, :, 0:RD // 2, 0]
                x1 = mv_[:, :, 0:RD // 2, 1]
                t0 = c_small.tile([128, HG, RD // 2], BF16, tag="t0")
                t1 = c_small.tile([128, HG, RD // 2], BF16, tag="t1")
                t2 = c_small.tile([128, HG, RD // 2], BF16, tag="t2")
                t3 = c_small.tile([128, HG, RD // 2], BF16, tag="t3")
                nc.vector.tensor_mul(out=t0, in0=x0, in1=cb)
                nc.vector.tensor_mul(out=t1, in0=x1, in1=sb_)
                nc.vector.tensor_mul(out=t2, in0=x0, in1=sb_)
                nc.vector.tensor_mul(out=t3, in0=x1, in1=cb)
                nc.gpsimd.tensor_sub(out=x0, in0=t0, in1=t1)
                nc.gpsimd.tensor_add(out=x1, in0=t2, in1=t3)

            # apply q rms reciprocal (k's is folded into the exp scale later)
            rrq_b = rrq.unsqueeze(2).broadcast_to([128, HG, HD])
            nc.vector.tensor_mul(out=qm.rearrange("p (h d) -> p h d", h=HG),
                                 in0=qm.rearrange("p (h d) -> p h d", h=HG),
                                 in1=rrq_b)

        c_small_cm.__exit__(None, None, None)
        c_cm.__exit__(None, None, None)
        avec_cm.__exit__(None, None, None)
        xnT_cm.__exit__(None, None, None)

        # ======== phases D/E/F share the left stack: oT under qkT ==================
        oT_cm = tc.tile_pool(name="oT", bufs=1, side="left")
        oT_p = oT_cm.__enter__()
        oT = oT_p.tile([128, HG, S], BF16)

        # ---- phase D: transpose q, k -> [hd, s] per head
        qkT_cm = tc.tile_pool(name="qkT", bufs=1, side="left")
        qkT_p = qkT_cm.__enter__()
        qT = qkT_p.tile([128, HG, S], BF16)
        kT = qkT_p.tile([128, HG, S], BF16)
        pst2_cm = tc.tile_pool(name="ps_tr2", bufs=4, space="PSUM")
        pst2 = pst2_cm.__enter__()
        for (nat, dst) in ((qnat, qT), (knat, kT)):
            for h in range(HG):
                for m in range(NT):
                    pt2 = pst2.tile([128, 128], BF16, tag="pt2")
                    nc.tensor.transpose(pt2, nat[:, m, h * 128:(h + 1) * 128], ident)
                    nc.vector.tensor_copy(out=dst[:, h, m * 128:(m + 1) * 128],
                                          in_=pt2)
        pst2_cm.__exit__(None, None, None)
        natqk_cm.__exit__(None, None, None)

        # ---- phase E: attention per head
        at_cm = tc.tile_pool(name="attn", bufs=3, side="left")
        at_p = at_cm.__enter__()
        rs_cm = tc.tile_pool(name="rsb", bufs=2, side="left")
        rs_p = rs_cm.__enter__()
        pssc_cm = tc.tile_pool(name="ps_sc", bufs=3, space="PSUM")
        pssc = pssc_cm.__enter__()
        pso_cm = tc.tile_pool(name="ps_o", bufs=1, space="PSUM")
        pso = pso_cm.__enter__()

        for h in range(HG):
            o_ps = pso.tile([128, S], F32, tag="o_ps")
            acc = rs_p.tile([128, S], BF16, tag="acc")
            for m in range(NT):
                sc = pssc.tile([128, S], F32, tag="sc")
                lhs_k = kT[:, h, m * 128:(m + 1) * 128]
                nc.tensor.matmul(sc[:, 0:512], lhs_k, qT[:, h, 0:512],
                                 start=True, stop=True)
                nc.tensor.matmul(sc[:, 512:1024], lhs_k, qT[:, h, 512:1024],
                                 start=True, stop=True)
                at = at_p.tile([128, S], BF16, tag="at", name="at")
                nc.scalar.activation(out=at, in_=sc, func=AF.Exp,
                                     scale=rrk_all[:, m, h:h + 1])
                # accumulate exp tiles on DVE (sums over the m-tiles)
                if m == 0:
                    nc.vector.tensor_copy(out=acc, in_=at)
                else:
                    nc.vector.tensor_add(out=acc, in0=acc, in1=at)
                first, last = (m == 0), (m == NT - 1)
                v_mh = vnat[:, m, h * 128:(h + 1) * 128]
                nc.tensor.matmul(o_ps[:, 0:512], v_mh, at[:, 0:512],
                                 start=first, stop=last)
                nc.tensor.matmul(o_ps[:, 512:1024], v_mh, at[:, 512:1024],
                                 start=first, stop=last)
            # sum over the sk partitions -> broadcast row, then normalize
            sums_b = rs_p.tile([128, S], F32, tag="sums_b")
            nc.gpsimd.partition_all_reduce(sums_b, acc, 128, bass_isa.ReduceOp.add)
            nc.vector.reciprocal(out=sums_b, in_=sums_b)
            nc.vector.tensor_mul(out=oT[:, h, :], in0=o_ps, in1=sums_b)

        pso_cm.__exit__(None, None, None)
        pssc_cm.__exit__(None, None, None)
        rs_cm.__exit__(None, None, None)
        at_cm.__exit__(None, None, None)
        qkT_cm.__exit__(None, None, None)
        v_cm.__exit__(None, None, None)

        # ---- phase F: out projection (transposed out)
        f_cm = tc.tile_pool(name="phF", bufs=3, side="left")
        f_p = f_cm.__enter__()
        psf_cm = tc.tile_pool(name="ps_out", bufs=2, space="PSUM")
        psf = psf_cm.__enter__()
        wo_r = wo.rearrange("(kb p) d -> p kb d", p=128)
        for m in range(KT):
            wo_t = f_p.tile([128, HG, 128], BF16, tag="wo_t")
            nc.sync.dma_start(out=wo_t, in_=wo_r[:, :, m * 128:(m + 1) * 128])
            po = psf.tile([128, S], F32, tag="po")
            for kb in range(HG):
                first, last = (kb == 0), (kb == HG - 1)
                nc.tensor.matmul(po[:, 0:512], wo_t[:, kb, :], oT[:, kb, 0:512],
                                 start=first, stop=last)
                nc.tensor.matmul(po[:, 512:1024], wo_t[:, kb, :],
                                 oT[:, kb, 512:1024],
                                 start=first, stop=last)
            ot_t = f_p.tile([128, S], F32, tag="ot_t")
            nc.scalar.activation(out=ot_t, in_=po, func=AF.Identity,
                                 bias=vb_sb[:, m:m + 1], scale=gate_sb[:, m:m + 1])
            nc.sync.dma_start(out=out_t[m * 128:(m + 1) * 128, :], in_=ot_t)
        psf_cm.__exit__(None, None, None)
        f_cm.__exit__(None, None, None)
        oT_cm.__exit__(None, None, None)
        misc_cm.__exit__(None, None, None)

    nc.compile()
    return nc


_NC_CACHE = {}


def _get_nc(has_qkv_bias, has_norm_w):
    key = (has_qkv_bias, has_norm_w)
    if key not in _NC_CACHE:
        _NC_CACHE[key] = build_nc(*key)
    return _NC_CACHE[key]


def prep_in_maps(x, mod, cos, sin, qkv_w, qkv_b, mod_w, mod_b, out_w, out_b,
                 norm_q_w, norm_k_w):
    """Host-side sharding. Returns (in_maps, flags, x_np)."""
    x = np.asarray(x, dtype=np.float32)
    m3 = np.asarray(mod, np.float32) @ np.asarray(mod_w, np.float32) \
        + np.asarray(mod_b, np.float32)
    bias, scale, gatef = np.split(m3, 3, axis=-1)          # [B, D] each
    scale1p = (1.0 + scale).astype(np.float32)
    vbf = (np.asarray(out_b, np.float32)[None, :] * gatef).astype(np.float32)

    qkv_b = np.asarray(qkv_b, np.float32)
    has_qkv_bias = bool(np.any(qkv_b != 0.0))
    has_norm_w = not (np.allclose(norm_q_w, 1.0) and np.allclose(norm_k_w, 1.0))

    import ml_dtypes
    bf16 = ml_dtypes.bfloat16
    cosc = np.ascontiguousarray(np.asarray(cos, np.float32).astype(bf16))
    sinc = np.ascontiguousarray(np.asarray(sin, np.float32).astype(bf16))
    qkv_w = np.asarray(qkv_w, np.float32).astype(bf16)
    out_w = np.asarray(out_w, np.float32).astype(bf16)

    in_maps = []
    for c in range(N_CORES):
        b, g = divmod(c, 2)
        lo = g * GCOLS
        im = {
            "x": np.ascontiguousarray(x[b]),
            "cos": cosc, "sin": sinc,
            "wq": np.ascontiguousarray(qkv_w[:, lo:lo + GCOLS]),
            "wk": np.ascontiguousarray(qkv_w[:, 2048 + lo:2048 + lo + GCOLS]),
            "wv": np.ascontiguousarray(qkv_w[:, 4096 + lo:4096 + lo + GCOLS]),
            "wo": np.ascontiguousarray(out_w[lo:lo + GCOLS, :]),
            "scale1p": np.ascontiguousarray(scale1p[b].reshape(KT, 128).T),
            "biasm": np.ascontiguousarray(bias[b].reshape(KT, 128).T),
            "gate": np.ascontiguousarray(gatef[b].reshape(KT, 128).T),
            "vb": np.ascontiguousarray(
                (vbf[b] if g == 0 else np.zeros_like(vbf[b])).reshape(KT, 128).T),
        }
        if has_qkv_bias:
            im["bq"] = np.ascontiguousarray(qkv_b[lo:lo + GCOLS].astype(bf16))
            im["bk"] = np.ascontiguousarray(
                qkv_b[2048 + lo:2048 + lo + GCOLS].astype(bf16))
            im["bv"] = np.ascontiguousarray(
                qkv_b[4096 + lo:4096 + lo + GCOLS].astype(bf16))
        if has_norm_w:
            im["wqn"] = np.ascontiguousarray(
                np.asarray(norm_q_w, np.float32).astype(bf16))
            im["wkn"] = np.ascontiguousarray(
                np.asarray(norm_k_w, np.float32).astype(bf16))
        in_maps.append(im)
    return in_maps, (has_qkv_bias, has_norm_w), x


def gather(results, x):
    B = x.shape[0]
    outs = []
    for b in range(B):
        p = results[2 * b]["out_t"] + results[2 * b + 1]["out_t"]   # [D, S]
        outs.append(p.T + x[b])
    return np.stack(outs).astype(np.float32)


def kernel(**inputs) -> np.ndarray:
    in_maps, flags, x = prep_in_maps(**inputs)
    nc = _get_nc(*flags)
    res = run_bass_kernel_spmd(nc, in_maps, core_ids=list(range(N_CORES)))
    return gather(res.results, x)


if __name__ == "__main__":
    import time
    t0 = time.time()
    nc = build_nc(False, False)
    print("build+compile ok in", time.time() - t0, "s")



# revision 42
# speedup vs baseline: 3.3136x; 1.0950x over previous
"""Trainium2 Bass kernel for ModalityAttention (B=4, S=1024, D=2048, H=16, HD=128, RD=64).

Sharding: 8 cores = 4 batches x 2 head-groups (8 heads each).
Each core computes, for its (batch b, head-group g):
  layernorm(x[b]) -> modulation (scale/bias precomputed on host from mod@mod_w)
  -> qkv projection for its 8 heads -> rmsnorm + rope -> attention
  -> partial out-projection (transposed layout) with gate folded in.
Host gathers: out[b] = (partial_g0 + partial_g1).T + x[b]
(residual added on host; vb = out_b*gate folded into the g0 partial on device).

Matmuls/transposes run in bf16 (weights cast on host); stats, softmax sums and
modulation vectors stay fp32.  Weight/cos/sin/output DMAs are consolidated into
a few large transfers (HWDGE issue overhead is ~0.6us per DMA).  The qkv
projection iterates s-tiles in the outer loop against a resident weight slab so
it can start as soon as the first layernormed s-tile is transposed, overlapping
phase A; q/k transposes are interleaved with the rms/rope loop so attention can
start early.  PSUM-evacuation copies rotate across ACT/DVE/Pool so no single
engine serializes a phase.
"""
import os, sys

for _p in ("/opt/trn_rl_repo", "/root/.axon_site/_ro/trn_rl_repo", "/root/.axon_site"):
    if os.path.isdir(_p) and _p not in sys.path:
        sys.path.insert(0, _p)

import numpy as np
import concourse.bass as bass
import concourse.bacc as bacc
import concourse.mybir as mybir
import concourse.tile as tile
from concourse import bass_isa
from concourse.masks import make_identity
from concourse.bass_utils import run_bass_kernel_spmd

F32 = mybir.dt.float32
BF16 = mybir.dt.bfloat16
AF = mybir.ActivationFunctionType
S, D, HG, HD, RD = 1024, 2048, 8, 128, 64
NT = S // 128        # 8 s-tiles
KT = D // 128        # 16 d-tiles
GCOLS = HG * HD      # 1024 columns per group per projection
EPS = 1e-6
N_CORES = 8


def _bcast_from_dram(ap, parts, reps=None):
    """DRAM AP -> partition-broadcast (and optional middle-dim repeat) source AP."""
    newap = [[0, parts]]
    if reps is not None:
        newap.append([0, reps])
    newap += list(ap.ap)
    return bass.AP(tensor=ap.tensor, offset=ap.offset, ap=newap)


def build_nc(has_qkv_bias: bool, has_norm_w: bool):
    nc = bacc.Bacc("TRN2", target_bir_lowering=False, debug=False,
                   enable_asserts=True, num_devices=N_CORES)

    x = nc.dram_tensor("x", [S, D], F32, kind="ExternalInput").ap()
    cos = nc.dram_tensor("cos", [S, RD // 2], BF16, kind="ExternalInput").ap()
    sin = nc.dram_tensor("sin", [S, RD // 2], BF16, kind="ExternalInput").ap()
    wq = nc.dram_tensor("wq", [D, GCOLS], BF16, kind="ExternalInput").ap()
    wk = nc.dram_tensor("wk", [D, GCOLS], BF16, kind="ExternalInput").ap()
    wv = nc.dram_tensor("wv", [D, GCOLS], BF16, kind="ExternalInput").ap()
    wo = nc.dram_tensor("wo", [GCOLS, D], BF16, kind="ExternalInput").ap()
    # modulation vectors, pre-reshaped on host to [128, KT] (column k = d-tile k)
    scale1p = nc.dram_tensor("scale1p", [128, KT], F32, kind="ExternalInput").ap()
    biasm = nc.dram_tensor("biasm", [128, KT], F32, kind="ExternalInput").ap()
    gate = nc.dram_tensor("gate", [128, KT], F32, kind="ExternalInput").ap()
    vb = nc.dram_tensor("vb", [128, KT], F32, kind="ExternalInput").ap()
    if has_qkv_bias:
        bq = nc.dram_tensor("bq", [GCOLS], BF16, kind="ExternalInput").ap()
        bk = nc.dram_tensor("bk", [GCOLS], BF16, kind="ExternalInput").ap()
        bv = nc.dram_tensor("bv", [GCOLS], BF16, kind="ExternalInput").ap()
    if has_norm_w:
        wqn = nc.dram_tensor("wqn", [HD], BF16, kind="ExternalInput").ap()
        wkn = nc.dram_tensor("wkn", [HD], BF16, kind="ExternalInput").ap()
    out_t = nc.dram_tensor("out_t", [D, S], F32, kind="ExternalOutput").ap()

    with tile.TileContext(nc) as tc:
        # ======== LEFT stack bottom: small persistent constants ====================
        misc_cm = tc.tile_pool(name="misc", bufs=1, side="left")
        misc = misc_cm.__enter__()
        ident = misc.tile([128, 128], BF16)
        make_identity(nc, ident)
        eps_t = misc.tile([128, 1], F32)
        nc.vector.memset(eps_t, EPS)
        eps128_t = misc.tile([128, 1], F32)
        nc.vector.memset(eps128_t, HD * EPS)
        gate_sb = misc.tile([128, KT], F32)
        vb_sb = misc.tile([128, KT], F32)
        rrk_all = misc.tile([128, NT, HG], F32)   # scaled k-rms reciprocals
        if has_norm_w:
            wqn_b = misc.tile([128, HG, HD], BF16)
            wkn_b = misc.tile([128, HG, HD], BF16)
        cs_c = misc.tile([128, NT, RD // 2], BF16)   # cos, s-tile m in dim 1
        cs_s = misc.tile([128, NT, RD // 2], BF16)

        # ======== RIGHT stack: wo slab (whole kernel) + natural qkv ===============
        wop_cm = tc.tile_pool(name="wopool", bufs=1, side="right")
        wop = wop_cm.__enter__()
        wo_sb = wop.tile([128, HG, D], BF16)   # wo[kb*128+p, d]
        v_cm = tc.tile_pool(name="vpool", bufs=1, side="right")
        v_p = v_cm.__enter__()
        vnat = v_p.tile([128, NT, GCOLS], BF16)
        natqk_cm = tc.tile_pool(name="natqk", bufs=1, side="right")
        natqk = natqk_cm.__enter__()
        qnat = natqk.tile([128, NT, GCOLS], BF16)
        knat = natqk.tile([128, NT, GCOLS], BF16)
        w_cm = tc.tile_pool(name="wstream", bufs=2, side="right")
        w_p = w_cm.__enter__()

        # weight slabs for the qkv projections: 6 groups of [128, KT, 512]
        wslabs = {}

        def emit_wslab(g):
            proj, n = divmod(g, 2)
            wdram = (wq, wk, wv)[proj]
            t = w_p.tile([128, KT, 512], BF16, tag="wslab")
            nc.sync.dma_start(
                out=t,
                in_=wdram.rearrange("(kb p) c -> p kb c", p=128)[
                    :, :, n * 512:(n + 1) * 512])
            wslabs[g] = t

        # ======== phase A: layernorm + modulation + transpose -> xnT ===============
        xnT_cm = tc.tile_pool(name="xnT", bufs=1, side="left")
        xnT_p = xnT_cm.__enter__()
        xnT = xnT_p.tile([128, KT, S], BF16)  # [d_in_tile, d_tile, s]

        avec_cm = tc.tile_pool(name="phA_vec", bufs=1, side="left")
        avec = avec_cm.__enter__()
        s1pc = avec.tile([128, KT], F32)
        bmc = avec.tile([128, KT], F32)
        if has_qkv_bias:
            bq_b = avec.tile([128, GCOLS], BF16)
            nc.sync.dma_start(out=bq_b, in_=_bcast_from_dram(bq, 128))
            bk_b = avec.tile([128, GCOLS], BF16)
            nc.sync.dma_start(out=bk_b, in_=_bcast_from_dram(bk, 128))
            bv_b = avec.tile([128, GCOLS], BF16)
            nc.sync.dma_start(out=bv_b, in_=_bcast_from_dram(bv, 128))

        a_cm = tc.tile_pool(name="phA", bufs=3, side="left")
        a_p = a_cm.__enter__()
        a_small_cm = tc.tile_pool(name="phA_small", bufs=4, side="left")
        a_small = a_small_cm.__enter__()
        pst_cm = tc.tile_pool(name="ps_tr", bufs=4, space="PSUM")
        pst = pst_cm.__enter__()

        for i in range(NT):
            xt = a_p.tile([128, D], F32, tag="xt")
            nc.sync.dma_start(out=xt, in_=x[i * 128:(i + 1) * 128, :])
            if i == 0:
                nc.sync.dma_start(out=s1pc, in_=scale1p)
                nc.sync.dma_start(out=bmc, in_=biasm)
                emit_wslab(0)
            if i == 2:
                emit_wslab(1)
            stats = a_small.tile([128, 4, 6], F32, tag="stats")
            xv = xt.rearrange("p (c f) -> p c f", c=4)
            for c in range(4):
                nc.vector.bn_stats(out=stats[:, c, :], in_=xv[:, c, :])
            mv = a_small.tile([128, 2], F32, tag="mv")
            nc.vector.bn_aggr(out=mv, in_=stats)
            rstd = a_small.tile([128, 1], F32, tag="rstd")
            nc.scalar.activation(out=rstd, in_=mv[:, 1:2], func=AF.Sqrt,
                                 bias=eps_t, scale=1.0)
            nc.vector.reciprocal(out=rstd, in_=rstd)
            nmr = a_small.tile([128, 1], F32, tag="nmr")
            nc.vector.tensor_mul(out=nmr, in0=mv[:, 0:1], in1=rstd)
            nc.scalar.mul(out=nmr, in_=nmr, mul=-1.0)
            xtb = a_p.tile([128, D], BF16, tag="xtb")
            nc.scalar.activation(out=xtb, in_=xt, func=AF.Identity,
                                 bias=nmr, scale=rstd)
            for k in range(KT):
                pt = pst.tile([128, 128], BF16, tag="pt")
                nc.tensor.transpose(pt, xtb[:, k * 128:(k + 1) * 128], ident)
                # modulation fused into the evac: xnT = pt * (1+scale[d]) + bias[d]
                # rotate ACT/DVE/Pool so no engine serializes the evacuation
                dst = xnT[:, k, i * 128:(i + 1) * 128]
                if k % 2 == 0:
                    nc.scalar.activation(out=dst, in_=pt, func=AF.Identity,
                                         bias=bmc[:, k:k + 1],
                                         scale=s1pc[:, k:k + 1])
                else:
                    nc.vector.tensor_scalar(
                        out=dst, in0=pt, scalar1=s1pc[:, k:k + 1],
                        scalar2=bmc[:, k:k + 1],
                        op0=mybir.AluOpType.mult, op1=mybir.AluOpType.add)

        # deferred misc loads (consumed in phases C/E/F) + wo slab prefetch
        nc.sync.dma_start(out=gate_sb, in_=gate)
        nc.sync.dma_start(out=vb_sb, in_=vb)
        if has_norm_w:
            nc.sync.dma_start(out=wqn_b, in_=_bcast_from_dram(wqn, 128, reps=HG))
            nc.sync.dma_start(out=wkn_b, in_=_bcast_from_dram(wkn, 128, reps=HG))
        nc.sync.dma_start(out=cs_c, in_=cos.rearrange("(m p) c -> p m c", p=128))
        nc.sync.dma_start(out=cs_s, in_=sin.rearrange("(m p) c -> p m c", p=128))
        nc.sync.dma_start(out=wo_sb, in_=wo.rearrange("(kb p) d -> p kb d", p=128))

        pst_cm.__exit__(None, None, None)
        a_small_cm.__exit__(None, None, None)
        a_cm.__exit__(None, None, None)

        # phase C/D pools opened BEFORE phase B emission so the rms/rope work and
        # the q/k transposes can overlap the tail of the qkv matmuls.
        c_cm = tc.tile_pool(name="phC", bufs=2, side="left")
        c_p = c_cm.__enter__()
        c_small_cm = tc.tile_pool(name="phC_small", bufs=2, side="left")
        c_small = c_small_cm.__enter__()
        qkT_cm = tc.tile_pool(name="qkT", bufs=1, side="left")
        qkT_p = qkT_cm.__enter__()
        qT = qkT_p.tile([128, HG, S], BF16)
        kT = qkT_p.tile([128, HG, S], BF16)
        pst2_cm = tc.tile_pool(name="ps_tr2", bufs=4, space="PSUM")
        pst2 = pst2_cm.__enter__()

        # ======== phase B: qkv projections (natural layout, m-outer) ==============
        psb_cm = tc.tile_pool(name="ps_qkv", bufs=4, space="PSUM")
        psb = psb_cm.__enter__()

        evac_rr = [nc.scalar, nc.vector]   # Pool/GPSIMD cannot read PSUM
        for g in range(6):
            proj, n = divmod(g, 2)
            nat = (qnat, knat, vnat)[proj]
            if g not in wslabs:
                emit_wslab(g)
            wslab = wslabs.pop(g)
            for m in range(NT):
                ps = psb.tile([128, 512], F32, tag="psb")
                for k in range(KT):
                    nc.tensor.matmul(ps, xnT[:, k, m * 128:(m + 1) * 128],
                                     wslab[:, k, :],
                                     start=(k == 0), stop=(k == KT - 1))
                eng = evac_rr[(g * NT + m) % 2]
                if eng is nc.scalar:
                    nc.scalar.copy(out=nat[:, m, n * 512:(n + 1) * 512], in_=ps)
                else:
                    eng.tensor_copy(out=nat[:, m, n * 512:(n + 1) * 512], in_=ps)
        if has_qkv_bias:
            for m in range(NT):
                nc.gpsimd.tensor_add(out=qnat[:, m, :], in0=qnat[:, m, :], in1=bq_b)
                nc.gpsimd.tensor_add(out=knat[:, m, :], in0=knat[:, m, :], in1=bk_b)
                nc.gpsimd.tensor_add(out=vnat[:, m, :], in0=vnat[:, m, :], in1=bv_b)

        psb_cm.__exit__(None, None, None)
        w_cm.__exit__(None, None, None)

        # ======== phase C+D: rmsnorm + rope, then per-head transposes =============
        for m in range(NT):
            qm = qnat[:, m, :]
            km = knat[:, m, :]
            ct = cs_c[:, m, :]
            st = cs_s[:, m, :]
            cb = ct.unsqueeze(1).broadcast_to([128, HG, RD // 2])
            sb_ = st.unsqueeze(1).broadcast_to([128, HG, RD // 2])

            # rms stats (on raw q/k, before norm-w and rope)
            sq = c_p.tile([128, GCOLS], BF16, tag="sqk")
            nc.vector.tensor_mul(out=sq, in0=qm, in1=qm)
            ssq = c_small.tile([128, HG], F32, tag="ssq")
            nc.vector.reduce_sum(out=ssq, in_=sq.rearrange("p (h d) -> p h d", h=HG),
                                 axis=mybir.AxisListType.X)
            rrq = c_small.tile([128, HG], F32, tag="rrq")
            nc.scalar.activation(out=rrq, in_=ssq, func=AF.Sqrt,
                                 bias=eps_t, scale=1.0 / HD)
            nc.vector.reciprocal(out=rrq, in_=rrq)

            sk_ = c_p.tile([128, GCOLS], BF16, tag="sqk")
            nc.vector.tensor_mul(out=sk_, in0=km, in1=km)
            ssk = c_small.tile([128, HG], F32, tag="ssk")
            nc.vector.reduce_sum(out=ssk, in_=sk_.rearrange("p (h d) -> p h d", h=HG),
                                 axis=mybir.AxisListType.X)
            nc.scalar.activation(out=rrk_all[:, m, :], in_=ssk, func=AF.Sqrt,
                                 bias=eps128_t, scale=1.0)
            nc.vector.reciprocal(out=rrk_all[:, m, :], in_=rrk_all[:, m, :])

            if has_norm_w:
                nc.vector.tensor_mul(out=qm.rearrange("p (h d) -> p h d", h=HG),
                                     in0=qm.rearrange("p (h d) -> p h d", h=HG),
                                     in1=wqn_b)
                nc.vector.tensor_mul(out=km.rearrange("p (h d) -> p h d", h=HG),
                                     in0=km.rearrange("p (h d) -> p h d", h=HG),
                                     in1=wkn_b)

            for mm in (qm, km):
                mv_ = mm.rearrange("p (h i two) -> p h i two", h=HG, two=2)
                x0 = mv_[:, :, 0:RD // 2, 0]
                x1 = mv_[:, :, 0:RD // 2, 1]
                t0 = c_small.tile([128, HG, RD // 2], BF16, tag="t0")
                t1 = c_small.tile([128, HG, RD // 2], BF16, tag="t1")
                t2 = c_small.tile([128, HG, RD // 2], BF16, tag="t2")
                t3 = c_small.tile([128, HG, RD // 2], BF16, tag="t3")
                nc.vector.tensor_mul(out=t0, in0=x0, in1=cb)
                nc.vector.tensor_mul(out=t1, in0=x1, in1=sb_)
                nc.vector.tensor_mul(out=t2, in0=x0, in1=sb_)
                nc.vector.tensor_mul(out=t3, in0=x1, in1=cb)
                nc.gpsimd.tensor_sub(out=x0, in0=t0, in1=t1)
                nc.gpsimd.tensor_add(out=x1, in0=t2, in1=t3)

            # apply q rms reciprocal (k's is folded into the exp scale later)
            rrq_b = rrq.unsqueeze(2).broadcast_to([128, HG, HD])
            nc.vector.tensor_mul(out=qm.rearrange("p (h d) -> p h d", h=HG),
                                 in0=qm.rearrange("p (h d) -> p h d", h=HG),
                                 in1=rrq_b)

            # ---- phase D fused: transpose q, k of this s-tile -> [hd, s]
            for (nat, dst) in ((qnat, qT), (knat, kT)):
                for h in range(HG):
                    pt2 = pst2.tile([128, 128], BF16, tag="pt2")
                    nc.tensor.transpose(pt2, nat[:, m, h * 128:(h + 1) * 128], ident)
                    eng = evac_rr[(m * HG + h) % 2]
                    to = dst[:, h, m * 128:(m + 1) * 128]
                    if eng is nc.scalar:
                        nc.scalar.copy(out=to, in_=pt2)
                    else:
                        eng.tensor_copy(out=to, in_=pt2)

        pst2_cm.__exit__(None, None, None)
        natqk_cm.__exit__(None, None, None)

        # ======== phase E: attention per head ======================================
        oT_cm = tc.tile_pool(name="oT", bufs=1, side="left")
        oT_p = oT_cm.__enter__()
        oT = oT_p.tile([128, HG, S], BF16)
        at_cm = tc.tile_pool(name="attn", bufs=3, side="left")
        at_p = at_cm.__enter__()
        rs_cm = tc.tile_pool(name="rsb", bufs=2, side="left")
        rs_p = rs_cm.__enter__()
        pssc_cm = tc.tile_pool(name="ps_sc", bufs=2, space="PSUM")
        pssc = pssc_cm.__enter__()
        pso_cm = tc.tile_pool(name="ps_o", bufs=2, space="PSUM")
        pso = pso_cm.__enter__()

        for h in range(HG):
            o_ps = pso.tile([128, S], F32, tag="o_ps")
            acc = rs_p.tile([128, S], BF16, tag="acc")
            for m in range(NT):
                sc = pssc.tile([128, S], F32, tag="sc")
                lhs_k = kT[:, h, m * 128:(m + 1) * 128]
                nc.tensor.matmul(sc[:, 0:512], lhs_k, qT[:, h, 0:512],
                                 start=True, stop=True)
                nc.tensor.matmul(sc[:, 512:1024], lhs_k, qT[:, h, 512:1024],
                                 start=True, stop=True)
                at = at_p.tile([128, S], BF16, tag="at", name="at")
                nc.scalar.activation(out=at, in_=sc, func=AF.Exp,
                                     scale=rrk_all[:, m, h:h + 1])
                # accumulate exp tiles on DVE (sums over the m-tiles)
                if m == 0:
                    nc.vector.tensor_copy(out=acc, in_=at)
                else:
                    nc.vector.tensor_add(out=acc, in0=acc, in1=at)
                first, last = (m == 0), (m == NT - 1)
                v_mh = vnat[:, m, h * 128:(h + 1) * 128]
                nc.tensor.matmul(o_ps[:, 0:512], v_mh, at[:, 0:512],
                                 start=first, stop=last)
                nc.tensor.matmul(o_ps[:, 512:1024], v_mh, at[:, 512:1024],
                                 start=first, stop=last)
            # sum over the sk partitions -> broadcast row, then normalize
            sums_b = rs_p.tile([128, S], F32, tag="sums_b")
            nc.gpsimd.partition_all_reduce(sums_b, acc, 128, bass_isa.ReduceOp.add)
            nc.vector.reciprocal(out=sums_b, in_=sums_b)
            nc.vector.tensor_mul(out=oT[:, h, :], in0=o_ps, in1=sums_b)

        pso_cm.__exit__(None, None, None)
        pssc_cm.__exit__(None, None, None)
        rs_cm.__exit__(None, None, None)
        at_cm.__exit__(None, None, None)
        v_cm.__exit__(None, None, None)

        # ======== phase F: out projection (transposed out) =========================
        f_cm = tc.tile_pool(name="phF", bufs=2, side="left")
        f_p = f_cm.__enter__()
        psf_cm = tc.tile_pool(name="ps_out", bufs=2, space="PSUM")
        psf = psf_cm.__enter__()
        for mb in range(KT // 4):
            stage = f_p.tile([128, 4, S], F32, tag="stage")
            for mm in range(4):
                m = mb * 4 + mm
                po = psf.tile([128, S], F32, tag="po")
                for kb in range(HG):
                    first, last = (kb == 0), (kb == HG - 1)
                    nc.tensor.matmul(po[:, 0:512],
                                     wo_sb[:, kb, m * 128:(m + 1) * 128],
                                     oT[:, kb, 0:512], start=first, stop=last)
                    nc.tensor.matmul(po[:, 512:1024],
                                     wo_sb[:, kb, m * 128:(m + 1) * 128],
                                     oT[:, kb, 512:1024], start=first, stop=last)
                nc.scalar.activation(out=stage[:, mm, :], in_=po, func=AF.Identity,
                                     bias=vb_sb[:, m:m + 1], scale=gate_sb[:, m:m + 1])
            nc.sync.dma_start(
                out=out_t[mb * 512:(mb + 1) * 512, :].rearrange(
                    "(mm p) s -> p mm s", p=128),
                in_=stage)
        psf_cm.__exit__(None, None, None)
        f_cm.__exit__(None, None, None)
        oT_cm.__exit__(None, None, None)
        qkT_cm.__exit__(None, None, None)
        c_small_cm.__exit__(None, None, None)
        c_cm.__exit__(None, None, None)
        avec_cm.__exit__(None, None, None)
        xnT_cm.__exit__(None, None, None)
        wop_cm.__exit__(None, None, None)
        misc_cm.__exit__(None, None, None)

    nc.compile()
    return nc


_NC_CACHE = {}


def _get_nc(has_qkv_bias, has_norm_w):
    key = (has_qkv_bias, has_norm_w)
    if key not in _NC_CACHE:
        _NC_CACHE[key] = build_nc(*key)
    return _NC_CACHE[key]


def prep_in_maps(x, mod, cos, sin, qkv_w, qkv_b, mod_w, mod_b, out_w, out_b,
                 norm_q_w, norm_k_w):
    """Host-side sharding. Returns (in_maps, flags, x_np)."""
    x = np.asarray(x, dtype=np.float32)
    m3 = np.asarray(mod, np.float32) @ np.asarray(mod_w, np.float32) \
        + np.asarray(mod_b, np.float32)
    bias, scale, gatef = np.split(m3, 3, axis=-1)          # [B, D] each
    scale1p = (1.0 + scale).astype(np.float32)
    vbf = (np.asarray(out_b, np.float32)[None, :] * gatef).astype(np.float32)

    qkv_b = np.asarray(qkv_b, np.float32)
    has_qkv_bias = bool(np.any(qkv_b != 0.0))
    has_norm_w = not (np.allclose(norm_q_w, 1.0) and np.allclose(norm_k_w, 1.0))

    import ml_dtypes
    bf16 = ml_dtypes.bfloat16
    cosc = np.ascontiguousarray(np.asarray(cos, np.float32).astype(bf16))
    sinc = np.ascontiguousarray(np.asarray(sin, np.float32).astype(bf16))
    qkv_w = np.asarray(qkv_w, np.float32).astype(bf16)
    out_w = np.asarray(out_w, np.float32).astype(bf16)

    in_maps = []
    for c in range(N_CORES):
        b, g = divmod(c, 2)
        lo = g * GCOLS
        im = {
            "x": np.ascontiguousarray(x[b]),
            "cos": cosc, "sin": sinc,
            "wq": np.ascontiguousarray(qkv_w[:, lo:lo + GCOLS]),
            "wk": np.ascontiguousarray(qkv_w[:, 2048 + lo:2048 + lo + GCOLS]),
            "wv": np.ascontiguousarray(qkv_w[:, 4096 + lo:4096 + lo + GCOLS]),
            "wo": np.ascontiguousarray(out_w[lo:lo + GCOLS, :]),
            "scale1p": np.ascontiguousarray(scale1p[b].reshape(KT, 128).T),
            "biasm": np.ascontiguousarray(bias[b].reshape(KT, 128).T),
            "gate": np.ascontiguousarray(gatef[b].reshape(KT, 128).T),
            "vb": np.ascontiguousarray(
                (vbf[b] if g == 0 else np.zeros_like(vbf[b])).reshape(KT, 128).T),
        }
        if has_qkv_bias:
            im["bq"] = np.ascontiguousarray(qkv_b[lo:lo + GCOLS].astype(bf16))
            im["bk"] = np.ascontiguousarray(
                qkv_b[2048 + lo:2048 + lo + GCOLS].astype(bf16))
            im["bv"] = np.ascontiguousarray(
                qkv_b[4096 + lo:4096 + lo + GCOLS].astype(bf16))
        if has_norm_w:
            im["wqn"] = np.ascontiguousarray(
                np.asarray(norm_q_w, np.float32).astype(bf16))
            im["wkn"] = np.ascontiguousarray(
                np.asarray(norm_k_w, np.float32).astype(bf16))
        in_maps.append(im)
    return in_maps, (has_qkv_bias, has_norm_w), x


def gather(results, x):
    B = x.shape[0]
    outs = []
    for b in range(B):
        p = results[2 * b]["out_t"] + results[2 * b + 1]["out_t"]   # [D, S]
        outs.append(p.T + x[b])
    return np.stack(outs).astype(np.float32)


def kernel(**inputs) -> np.ndarray:
    in_maps, flags, x = prep_in_maps(**inputs)
    nc = _get_nc(*flags)
    res = run_bass_kernel_spmd(nc, in_maps, core_ids=list(range(N_CORES)))
    return gather(res.results, x)


if __name__ == "__main__":
    import time
    t0 = time.time()
    nc = build_nc(False, False)
    print("build+compile ok in", time.time() - t0, "s")
